# revision 1
# baseline (speedup 1.0000x reference)
"""Trainium2 Bass kernel for masked-GRU + residual + LayerNorm.

Problem: N=128 sequences of length L=512, hidden H=512.
  gx = x @ W_ih.T + b_ih            (precomputable input projection)
  per step l: hc = h * (1-is_initial[l]); gh = hc @ W_hh.T + b_hh
    r = sig(gx_r+gh_r); z = sig(gx_z+gh_z); n = tanh(gx_n + r*gh_n)
    h = (1-z)*n + z*hc
  out = LayerNorm(seq + x) * gamma + beta;  h_exp = broadcast(h_last)

Strategy:
  * Data parallel: 16 batch rows per core (8 cores).
  * Sequence-chunk parallel: each L=512 sequence is split into C=16
    chunks of 32 steps. Chunks are processed as independent columns,
    made exact by an R-step warm-up: the state entering chunk c only
    depends on inputs back to the latest reset (is_initial==1), and the
    data has a reset within every 16-step window (verified at runtime;
    R grows if needed). Chunk 0 injects the true h0 at its first step.
  * Per-core per-step state: hT [H=512 (4 partition tiles), 256 cols].
    Weights stationary (f32r = full PE speed), moving free dim 256.
    r/z gates: W_hh and W_ih matmuls accumulate into the same PSUM,
    sigmoid applied straight out of PSUM with fused bias.
  * LayerNorm over H (partition dim) via ones-vector matmul reductions,
    batched over blocks of 4 steps; mean/rstd broadcast back across
    partitions with a DRAM-bounce broadcast DMA.
  * h_exp is h_last broadcast over L: computed on device as h_last,
    replicated during the host-side unshard.
"""
import sys

sys.path.insert(0, "/opt/trn_rl_repo")

import numpy as np

import concourse.bass as bass
import concourse.tile as tile
from concourse import bacc, mybir
from concourse.bass_utils import run_bass_kernel_spmd

F32 = mybir.dt.float32
F32R = mybir.dt.float32r
AF = mybir.ActivationFunctionType
ALU = mybir.AluOpType

N, L, H = 128, 512, 512
NCORES = 8
NB = N // NCORES          # batch rows per core = 16
C = 16                    # chunks per sequence
KS = L // C               # main steps per chunk = 32
S = NB * C                # columns per core = 256
HT = H // 128             # h partition tiles = 4
GT = 3 * H // 128         # gate partition tiles = 12
BLK = 4                   # LN block (main steps)
NBLK = KS // BLK          # 8


def _bcast_ap(row_ap, parts=128):
    """DRAM row AP -> partition-broadcast AP (step 0 over partitions)."""
    return bass.AP(
        tensor=row_ap.tensor,
        offset=row_ap.offset,
        ap=[[0, parts]] + [list(d) for d in row_ap.ap],
    )


def build_program(R=16, triv_gb=False):
    T = R + KS
    nc = bacc.Bacc("TRN2", target_bir_lowering=False)

    xs_d = nc.declare_dram_parameter("xs", [HT, 128, T, S], F32R, isOutput=False)
    ms_d = nc.declare_dram_parameter("ms", [T, S], F32, isOutput=False)
    h0m_d = nc.declare_dram_parameter("h0m", [HT, 128, NB], F32R, isOutput=False)
    wih_d = nc.declare_dram_parameter("wih", [HT, 128, 3 * H], F32R, isOutput=False)
    whh_d = nc.declare_dram_parameter("whh", [HT, 128, 3 * H], F32R, isOutput=False)
    brz_d = nc.declare_dram_parameter("brz", [128, 8], F32, isOutput=False)
    bhn_d = nc.declare_dram_parameter("bhn", [128, HT], F32, isOutput=False)
    bin_d = nc.declare_dram_parameter("bin", [128, HT], F32, isOutput=False)
    gam_d = nc.declare_dram_parameter("gam", [128, HT], F32, isOutput=False)
    bet_d = nc.declare_dram_parameter("bet", [128, HT], F32, isOutput=False)
    ones_d = nc.declare_dram_parameter("ones", [128, 1], F32R, isOutput=False)
    zeros_d = nc.declare_dram_parameter("zeros", [128, S], F32R, isOutput=False)

    out_d = nc.declare_dram_parameter("out_st", [HT, 128, KS, S], F32, isOutput=True)
    hl_d = nc.declare_dram_parameter("hlast", [HT, 128, NB], F32, isOutput=True)

    scr = nc.dram_tensor("lnscr", [NBLK, 2048], F32)

    with tile.TileContext(nc) as tc:
        with (
            tc.tile_pool(name="const", bufs=1) as cst,
            tc.tile_pool(name="sb", bufs=1) as sb,
            tc.tile_pool(name="rp", bufs=4, space="PSUM") as rp,
            tc.tile_pool(name="ip", bufs=4, space="PSUM") as ip,
        ):
            # ---- constants (wih first: needed by the t=0 prefill) ----
            wih_sb, whh_sb, h0m_sb = [], [], []
            for k in range(HT):
                w1 = cst.tile([128, 3 * H], F32R, name=f"wih_sb{k}", tag=f"wih{k}")
                nc.sync.dma_start(out=w1, in_=wih_d[k, :, :])
                wih_sb.append(w1)
            x0 = []
            for k in range(HT):
                x1 = sb.tile([128, S], F32R, name=f"xt0_{k}", tag="xt", bufs=8)
                nc.sync.dma_start(out=x1, in_=xs_d[k, :, 0, :])
                x0.append(x1)
            for k in range(HT):
                w2 = cst.tile([128, 3 * H], F32R, name=f"whh_sb{k}", tag=f"whh{k}")
                nc.sync.dma_start(out=w2, in_=whh_d[k, :, :])
                whh_sb.append(w2)
                hm = cst.tile([128, NB], F32R, name=f"h0m_sb{k}", tag=f"h0m{k}")
                nc.sync.dma_start(out=hm, in_=h0m_d[k, :, :])
                h0m_sb.append(hm)
            brz_sb = cst.tile([128, 8], F32, name="brz_sb", tag="brz")
            nc.sync.dma_start(out=brz_sb, in_=brz_d[:, :])
            bhn_sb = cst.tile([128, HT], F32, name="bhn_sb", tag="bhn")
            nc.sync.dma_start(out=bhn_sb, in_=bhn_d[:, :])
            bin_sb = cst.tile([128, HT], F32, name="bin_sb", tag="bin")
            nc.sync.dma_start(out=bin_sb, in_=bin_d[:, :])
            gam_sb = cst.tile([128, HT], F32, name="gam_sb", tag="gam")
            nc.sync.dma_start(out=gam_sb, in_=gam_d[:, :])
            bet_sb = cst.tile([128, HT], F32, name="bet_sb", tag="bet")
            nc.sync.dma_start(out=bet_sb, in_=bet_d[:, :])
            ones_sb = cst.tile([128, 1], F32R, name="ones_sb", tag="ones")
            nc.sync.dma_start(out=ones_sb, in_=ones_d[:, :])
            eps_sb = cst.tile([1, 1], F32, name="eps_sb", tag="eps")
            nc.vector.memset(eps_sb, 1e-5)

            # ---- initial (zero) state ----
            s_cur = []
            for k in range(HT):
                st = sb.tile([128, S], F32R, name=f"s_init{k}", tag="state", bufs=8)
                nc.sync.dma_start(out=st, in_=zeros_d[:, :])
                s_cur.append(st)

            out_flat = [out_d[k, :, :, :].rearrange("p t s -> p (t s)") for k in range(HT)]

            def load_x(t):
                xt = []
                for k in range(HT):
                    x1 = sb.tile([128, S], F32R, name=f"xt{t}_{k}", tag="xt", bufs=8)
                    nc.sync.dma_start(out=x1, in_=xs_d[k, :, t, :])
                    xt.append(x1)
                return xt

            def prefill_gxn(t, xt):
                # complete psum groups for gx_n of step t (wih only),
                # drained straight to SBUF on ACT with b_in folded in
                gx_ps = [
                    ip.tile([128, 512], F32, name=f"gx{t}_{j}", tag="ip")
                    for j in range(2)
                ]
                for k4 in range(4):
                    j = 8 + k4
                    oap = gx_ps[k4 // 2][:, (k4 % 2) * 256 : (k4 % 2) * 256 + 256]
                    for k in range(HT):
                        nc.tensor.matmul(
                            oap, wih_sb[k][:, j * 128 : (j + 1) * 128], xt[k],
                            start=(k == 0), stop=(k == HT - 1))
                gxs = []
                for k in range(HT):
                    g1 = sb.tile([128, S], F32, name=f"gxs{t}_{k}", tag="gxs", bufs=4)
                    nc.scalar.activation(
                        out=g1,
                        in_=gx_ps[k // 2][:, (k % 2) * 256 : (k % 2) * 256 + 256],
                        func=AF.Identity, bias=bin_sb[:, k : k + 1], scale=1.0)
                    gxs.append(g1)
                return gxs

            def prefill_r(t, xt):
                # open accumulation groups for the r gate of step t
                # (one full PSUM bank per r g-tile; closed by whh next step)
                r_ps = [
                    rp.tile([128, 256], F32, name=f"r{t}_{j}", tag="rp")
                    for j in range(4)
                ]
                for j in range(4):
                    for k in range(HT):
                        nc.tensor.matmul(
                            r_ps[j], wih_sb[k][:, j * 128 : (j + 1) * 128], xt[k],
                            start=(k == 0), stop=False)
                return r_ps

            xt = x0
            gxs_cur = prefill_gxn(0, xt)
            r_ps = prefill_r(0, xt)

            y_blk = None
            for t in range(T):
                main = t >= R
                toff = (t - R) % BLK
                blk = (t - R) // BLK

                if t + 1 < T:
                    xt_nxt = load_x(t + 1)
                    mk = sb.tile([128, S], F32, name=f"mk{t}", tag="mask", bufs=3)
                    nc.scalar.dma_start(out=mk, in_=_bcast_ap(ms_d[t + 1, :]))

                # -- close r groups with the recurrent part --
                # k-outer: the first matmuls need only s_cur[0], so PE can
                # start as soon as the first state tile is masked
                for k in range(HT):
                    for j in range(4):
                        nc.tensor.matmul(
                            r_ps[j], whh_sb[k][:, j * 128 : (j + 1) * 128], s_cur[k],
                            start=False, stop=(k == HT - 1))
                # -- gh_n (whh only, complete groups) --
                gh_ps = [
                    ip.tile([128, 512], F32, name=f"gh{t}_{j}", tag="ip")
                    for j in range(2)
                ]
                for k4 in range(4):
                    j = 8 + k4
                    oap = gh_ps[k4 // 2][:, (k4 % 2) * 256 : (k4 % 2) * 256 + 256]
                    for k in range(HT):
                        nc.tensor.matmul(
                            oap, whh_sb[k][:, j * 128 : (j + 1) * 128], s_cur[k],
                            start=(k == 0), stop=(k == HT - 1))
                # -- z gate (whh + wih complete groups, in-step) --
                z_ps = [
                    ip.tile([128, 512], F32, name=f"z{t}_{j}", tag="ip")
                    for j in range(2)
                ]
                for j4 in range(4):
                    j = 4 + j4
                    oap = z_ps[j4 // 2][:, (j4 % 2) * 256 : (j4 % 2) * 256 + 256]
                    for k in range(HT):
                        nc.tensor.matmul(
                            oap, whh_sb[k][:, j * 128 : (j + 1) * 128], s_cur[k],
                            start=(k == 0), stop=False)
                    for k in range(HT):
                        nc.tensor.matmul(
                            oap, wih_sb[k][:, j * 128 : (j + 1) * 128], xt[k],
                            start=False, stop=(k == HT - 1))

                # -- prefill next step (PE stays busy during gate math) --
                if t + 1 < T:
                    gxs_nxt = prefill_gxn(t + 1, xt_nxt)
                    r_nxt = prefill_r(t + 1, xt_nxt)

                # -- sigmoids straight from PSUM (bias = b_ih + b_hh) --
                r_t, z_t = [], []
                for k in range(HT):
                    rt = sb.tile([128, S], F32, name=f"rt{t}_{k}", tag="rt", bufs=6)
                    nc.scalar.activation(
                        out=rt, in_=r_ps[k],
                        func=AF.Sigmoid, bias=brz_sb[:, k : k + 1], scale=1.0)
                    r_t.append(rt)
                for k in range(HT):
                    j = 4 + k
                    zt = sb.tile([128, S], F32, name=f"zt{t}_{k}", tag="zt", bufs=6)
                    nc.scalar.activation(
                        out=zt, in_=z_ps[k // 2][:, (k % 2) * 256 : (k % 2) * 256 + 256],
                        func=AF.Sigmoid, bias=brz_sb[:, j : j + 1], scale=1.0)
                    z_t.append(zt)
                # -- n gate --
                gxs = gxs_cur
                n_t = []
                for k in range(HT):
                    stt = sb.tile([128, S], F32, name=f"st{t}_{k}", tag="stt", bufs=4)
                    nc.vector.scalar_tensor_tensor(
                        out=stt,
                        in0=gh_ps[k // 2][:, (k % 2) * 256 : (k % 2) * 256 + 256],
                        scalar=bhn_sb[:, k : k + 1], in1=r_t[k],
                        op0=ALU.add, op1=ALU.mult)
                    u = sb.tile([128, S], F32, name=f"u{t}_{k}", tag="u", bufs=6)
                    nc.vector.tensor_add(u, stt, gxs[k])
                    nt = sb.tile([128, S], F32, name=f"nt{t}_{k}", tag="nt", bufs=6)
                    nc.scalar.activation(
                        out=nt, in_=u, func=AF.Tanh, scale=1.0)
                    n_t.append(nt)
                # -- hidden update: hn = (s - n)*z + n --
                hn = []
                for k in range(HT):
                    t1 = sb.tile([128, S], F32, name=f"t1{t}_{k}", tag="t1", bufs=4)
                    nc.gpsimd.tensor_sub(t1, s_cur[k], n_t[k])
                    t2 = sb.tile([128, S], F32, name=f"t2{t}_{k}", tag="t2", bufs=4)
                    nc.vector.tensor_mul(t2, t1, z_t[k])
                    hh = sb.tile([128, S], F32, name=f"hn{t}_{k}", tag="hn", bufs=8)
                    nc.vector.tensor_add(hh, t2, n_t[k])
                    hn.append(hh)

                # -- residual into LN block buffer --
                if main:
                    if toff == 0:
                        y_blk = [
                            sb.tile([128, BLK * S], F32R, name=f"yb{blk}_{k}",
                                    tag=f"yb{k}", bufs=2)
                            for k in range(HT)
                        ]
                        y2 = [
                            sb.tile([128, BLK * S], F32R, name=f"y2_{blk}_{k}",
                                    tag="y2", bufs=1)
                            for k in range(HT)
                        ]
                    yadd_eng = nc.vector if toff == BLK - 1 else nc.gpsimd
                    for k in range(HT):
                        ysl = y_blk[k][:, toff * S : (toff + 1) * S]
                        yadd_eng.tensor_add(ysl, hn[k], xt[k])
                        nc.gpsimd.tensor_mul(
                            y2[k][:, toff * S : (toff + 1) * S], ysl, ysl)

                # -- next state (masked), h0 injection at entry to main --
                if t + 1 < T:
                    s_nxt = []
                    for k in range(HT):
                        sn = sb.tile([128, S], F32R, name=f"s{t + 1}_{k}",
                                     tag="state", bufs=8)
                        nc.vector.tensor_mul(sn, hn[k], mk)
                        s_nxt.append(sn)
                    if t + 1 == R:
                        for k in range(HT):
                            nc.vector.tensor_copy(
                                s_nxt[k][:, 0:S:C], h0m_sb[k])
                    s_cur = s_nxt
                    xt = xt_nxt
                    gxs_cur = gxs_nxt
                    r_ps = r_nxt

                # -- LayerNorm for a finished block of 4 steps --
                if main and toff == BLK - 1:
                    FB = BLK * S  # 1024
                    mu_ps = [
                        ip.tile([1, 512], F32, name=f"mu{blk}_{h}", tag="ip")
                        for h in range(2)
                    ]
                    ss_ps = [
                        ip.tile([1, 512], F32, name=f"ss{blk}_{h}", tag="ip")
                        for h in range(2)
                    ]
                    for half in range(2):
                        for k in range(HT):
                            nc.tensor.matmul(
                                mu_ps[half], ones_sb,
                                y_blk[k][:, half * 512 : (half + 1) * 512],
                                start=(k == 0), stop=(k == HT - 1))
                        for k in range(HT):
                            nc.tensor.matmul(
                                ss_ps[half], ones_sb,
                                y2[k][:, half * 512 : (half + 1) * 512],
                                start=(k == 0), stop=(k == HT - 1))
                    mu_s = sb.tile([1, FB], F32, name=f"mus{blk}", tag="mus", bufs=2)
                    ss_s = sb.tile([1, FB], F32, name=f"sss{blk}", tag="sss", bufs=2)
                    for half in range(2):
                        nc.vector.tensor_scalar_mul(
                            mu_s[:, half * 512 : (half + 1) * 512], mu_ps[half],
                            1.0 / H)
                        nc.vector.tensor_scalar_mul(
                            ss_s[:, half * 512 : (half + 1) * 512], ss_ps[half],
                            1.0 / H)
                    var_s = sb.tile([1, FB], F32, name=f"var{blk}", tag="vars", bufs=2)
                    nc.vector.scalar_tensor_tensor(
                        out=var_s, in0=mu_s, scalar=-1.0, in1=mu_s,
                        op0=ALU.mult, op1=ALU.mult)
                    nc.vector.tensor_add(var_s, var_s, ss_s)
                    std_s = sb.tile([1, FB], F32, name=f"std{blk}", tag="stds", bufs=2)
                    nc.scalar.activation(
                        out=std_s, in_=var_s, func=AF.Sqrt, bias=eps_sb, scale=1.0)
                    rst_s = sb.tile([1, FB], F32, name=f"rst{blk}", tag="rsts", bufs=2)
                    nc.vector.reciprocal_approx_fast(out=rst_s, in_=std_s)
                    nc.scalar.dma_start(out=scr[blk : blk + 1, 0:1024], in_=mu_s)
                    nc.scalar.dma_start(out=scr[blk : blk + 1, 1024:2048], in_=rst_s)
                    mu_bc = sb.tile([128, FB], F32, name=f"mubc{blk}", tag="mubc", bufs=1)
                    rs_bc = sb.tile([128, FB], F32, name=f"rsbc{blk}", tag="rsbc", bufs=1)
                    nc.scalar.dma_start(out=mu_bc, in_=_bcast_ap(scr[blk, 0:1024]))
                    nc.scalar.dma_start(out=rs_bc, in_=_bcast_ap(scr[blk, 1024:2048]))
                    norm_eng = nc.vector if blk == NBLK - 1 else nc.gpsimd
                    for k in range(HT):
                        yn = sb.tile([128, FB], F32, name=f"yn{blk}_{k}", tag="yn", bufs=2)
                        norm_eng.tensor_sub(yn, y_blk[k], mu_bc)
                        norm_eng.tensor_mul(yn, yn, rs_bc)
                        if not triv_gb:
                            nc.vector.tensor_scalar(
                                out=yn, in0=yn,
                                scalar1=gam_sb[:, k : k + 1],
                                scalar2=bet_sb[:, k : k + 1],
                                op0=ALU.mult, op1=ALU.add)
                        nc.sync.dma_start(
                            out=out_flat[k][:, blk * FB : (blk + 1) * FB], in_=yn)

                # -- final hidden state (chunk C-1 columns) --
                if t == T - 1:
                    for k in range(HT):
                        nc.sync.dma_start(
                            out=hl_d[k, :, :], in_=hn[k][:, C - 1 : S : C])
    nc.compile()
    return nc


def stage_inputs(input, h, is_initial, W_ih, W_hh, b_ih, b_hh, gamma, beta, R):
    """Host-side sharding/staging. Returns per-core input maps."""
    T = R + KS
    x = np.asarray(input, np.float32)
    h0 = np.asarray(h, np.float32)
    ii = np.asarray(is_initial).reshape(N, L)
    W_ih = np.asarray(W_ih, np.float32)
    W_hh = np.asarray(W_hh, np.float32)
    b_ih = np.asarray(b_ih, np.float32)
    b_hh = np.asarray(b_hh, np.float32)
    gamma = np.asarray(gamma, np.float32)
    beta = np.asarray(beta, np.float32)

    mask = 1.0 - ii.astype(np.float32)  # [N, L]

    # l index per (c, t): warm-up reads the R steps before the chunk;
    # chunk 0's warm-up reads l in [KS-R, KS) (discarded garbage).
    l_for = np.empty((C, T), np.int64)
    for c in range(C):
        for t in range(T):
            l = c * KS + (t - R)
            l_for[c, t] = l if l >= 0 else l + KS
    # masks per phase step t: state entering step t is h * ms[t]
    # ms[t, s] with s = n*C + c uses mask at l_for[c, t]
    # (for t=0 it is unused; state starts at zero)

    # weight layouts: wih[k, p, g] = W_ih[g, k*128+p]
    wihT = np.ascontiguousarray(
        W_ih.T.reshape(HT, 128, 3 * H))
    whhT = np.ascontiguousarray(
        W_hh.T.reshape(HT, 128, 3 * H))
    brz = (b_ih + b_hh)[: 2 * H].reshape(8, 128).T.copy()        # [128, 8]
    bhn = b_hh[2 * H :].reshape(HT, 128).T.copy()                # [128, 4]
    binn = b_ih[2 * H :].reshape(HT, 128).T.copy()
    gam = gamma.reshape(HT, 128).T.copy()
    bet = beta.reshape(HT, 128).T.copy()
    ones = np.ones((128, 1), np.float32)
    zeros = np.zeros((128, S), np.float32)

    in_maps = []
    for core in range(NCORES):
        n0 = core * NB
        xc = x[n0 : n0 + NB]              # [NB, L, H]
        # xs[k, p, t, s] = x[n, l_for[c, t], k*128+p], s = n*C + c
        xg = xc[:, l_for, :]              # [NB, C, T, H]
        xs = np.ascontiguousarray(
            xg.transpose(3, 2, 0, 1).reshape(HT, 128, T, S))
        mg = mask[n0 : n0 + NB][:, l_for]  # [NB, C, T]
        ms = np.ascontiguousarray(mg.transpose(2, 0, 1).reshape(T, S))
        m0 = mask[n0 : n0 + NB, 0]         # [NB]
        h0m = np.ascontiguousarray(
            (h0[n0 : n0 + NB] * m0[:, None]).T.reshape(HT, 128, NB))
        in_maps.append({
            "xs": xs, "ms": ms, "h0m": h0m,
            "wih": wihT, "whh": whhT, "brz": brz, "bhn": bhn, "bin": binn,
            "gam": gam, "bet": bet, "ones": ones, "zeros": zeros,
        })
    return in_maps


def required_warmup(is_initial):
    """Max distance from a chunk boundary back to the latest reset."""
    ii = np.asarray(is_initial).reshape(N, L)
    need = 0
    for c in range(1, C):
        start = c * KS
        sub = ii[:, :start]
        for n in range(N):
            nz = np.nonzero(sub[n])[0]
            gap = start - nz[-1] if len(nz) else start
            need = max(need, gap)
    return need


def unstage_outputs(results):
    out = np.empty((N, L, H), np.float32)
    h_last = np.empty((N, H), np.float32)
    for core in range(NCORES):
        n0 = core * NB
        st = results[core]["out_st"]      # [HT, 128, KS, S]
        o = st.reshape(HT, 128, KS, NB, C).transpose(3, 4, 2, 0, 1)
        out[n0 : n0 + NB] = o.reshape(NB, L, H)
        hl = results[core]["hlast"]       # [HT, 128, NB]
        h_last[n0 : n0 + NB] = hl.transpose(2, 0, 1).reshape(NB, H)
    h_exp = np.broadcast_to(h_last[:, None, :], (N, L, H)).copy()
    return out, h_exp


_PROGRAM_CACHE = {}


def kernel(input, h, is_initial, W_ih, W_hh, b_ih, b_hh, gamma, beta):
    need = required_warmup(is_initial)
    R = 12
    while R < need:
        R += 4
    triv = bool(
        np.all(np.asarray(gamma) == 1.0) and np.all(np.asarray(beta) == 0.0))
    key = (R, triv)
    if key not in _PROGRAM_CACHE:
        _PROGRAM_CACHE[key] = build_program(R, triv_gb=triv)
    nc = _PROGRAM_CACHE[key]
    in_maps = stage_inputs(
        input, h, is_initial, W_ih, W_hh, b_ih, b_hh, gamma, beta, R)
    res = run_bass_kernel_spmd(nc, in_maps, list(range(NCORES))).results
    return unstage_outputs(res)



# revision 26
# speedup vs baseline: 1.0669x; 1.0669x over previous
"""Trainium2 Bass kernel for masked-GRU + residual + LayerNorm.

Problem: N=128 sequences of length L=512, hidden H=512.
  gx = x @ W_ih.T + b_ih            (precomputable input projection)
  per step l: hc = h * (1-is_initial[l]); gh = hc @ W_hh.T + b_hh
    r = sig(gx_r+gh_r); z = sig(gx_z+gh_z); n = tanh(gx_n + r*gh_n)
    h = (1-z)*n + z*hc
  out = LayerNorm(seq + x) * gamma + beta;  h_exp = broadcast(h_last)

Strategy (v2):
  * Data parallel: 16 batch rows per core (8 cores).
  * Sequence-chunk parallel: each L=512 sequence is split into C=16
    chunks of 32 steps, processed as independent columns, made exact by
    an R-step warm-up (a reset lands inside every R-window; verified at
    runtime). Chunk 0 injects the true h0 at entry to main.
  * bf16 everywhere off-PSUM: weights, x, gates, state, y, outputs.
    PSUM stays f32.  DVE runs 2-byte all-SBUF ops at 4x rate.
  * Packed state s [128, 1024] bf16 (4 h-tiles side by side in the free
    dim) so the elementwise chain runs as [128,512] halves.
  * PSUM banks (8): R pair (r gates), Z pair (z), G pair (gh_n),
    X pair (gx_n, lives across one step boundary).  LN stats borrow
    bank G0 right after it drains.
  * n-gate: gx_n stays in PSUM; w=(gh+bhh_n)*r on DVE, u=w+gx on Pool,
    tanh(u)+b_ih_n on ACT.  No identity-drain of gx.
  * LayerNorm per block of 8 steps, software-pipelined over the 7
    following steps so nothing blocks the recurrence: column sums via
    indicator-stationary matmuls into ONE [8,512] PSUM tile (rows 0-3
    Smu, rows 4-7 Sss), stats math as [4,512] multi-partition ops
    (D = H*Sss - Smu^2; rs = 1/sqrt(D + eps*H^2); out = ((H*y - Smu)
    * rs) * gamma + beta), DRAM-bounce broadcast of (Smu, rs) in bf16.
  * out / h_last stored bf16, upcast host-side.
"""
import sys

sys.path.insert(0, "/opt/trn_rl_repo")

import numpy as np

import concourse.bass as bass
import concourse.tile as tile
from concourse import bacc, mybir
from concourse.bass_utils import run_bass_kernel_spmd

F32 = mybir.dt.float32
BF16 = mybir.dt.bfloat16
AF = mybir.ActivationFunctionType
ALU = mybir.AluOpType

N, L, H = 128, 512, 512
NCORES = 8
NB = N // NCORES          # batch rows per core = 16
C = 16                    # chunks per sequence
KS = L // C               # main steps per chunk = 32
S = NB * C                # columns per core = 256
HT = H // 128             # h partition tiles = 4
BLK = 8                   # LN block (main steps)
NBLK = KS // BLK          # 4
FB = BLK * S              # 2048 block columns
SF = HT * S               # 1024 free dim of the packed state


def _bcast_ap(row_ap, parts=128, rep=1):
    """DRAM row AP -> partition-broadcast AP (0-stride over partitions,
    optionally replicated rep times along free)."""
    ap = [[0, parts]]
    if rep > 1:
        ap.append([0, rep])
    ap += [list(d) for d in row_ap.ap]
    return bass.AP(tensor=row_ap.tensor, offset=row_ap.offset, ap=ap)


def build_program(R=12, debug=False):
    T = R + KS
    nc = bacc.Bacc("TRN2", target_bir_lowering=False)

    xs_d = nc.declare_dram_parameter("xs", [T, 128, SF], BF16, isOutput=False)
    ms_d = nc.declare_dram_parameter("ms", [T, S], BF16, isOutput=False)
    h0m_d = nc.declare_dram_parameter("h0m", [128, HT * NB], BF16, isOutput=False)
    wih_d = nc.declare_dram_parameter("wih", [HT, 128, 3 * H], BF16, isOutput=False)
    whh_d = nc.declare_dram_parameter("whh", [HT, 128, 3 * H], BF16, isOutput=False)
    brz_d = nc.declare_dram_parameter("brz", [128, 8], F32, isOutput=False)
    bhn_d = nc.declare_dram_parameter("bhn", [128, HT], F32, isOutput=False)
    bin_d = nc.declare_dram_parameter("bin", [128, HT], F32, isOutput=False)
    gam_d = nc.declare_dram_parameter("gam", [128, HT], F32, isOutput=False)
    bet_d = nc.declare_dram_parameter("bet", [128, HT], F32, isOutput=False)
    ind_d = nc.declare_dram_parameter("ind", [128, 16], BF16, isOutput=False)

    out_d = nc.declare_dram_parameter("out_st", [HT, 128, KS * S], BF16, isOutput=True)
    if debug:
        dbg_names = ["mk0", "r0", "z0", "n0", "hn0", "s1", "s2x", "hnT"]
        dbg_d = {
            nm: nc.declare_dram_parameter(f"dbg_{nm}", [128, SF], BF16,
                                          isOutput=True)
            for nm in dbg_names
        }
    hl_d = nc.declare_dram_parameter("hlast", [128, HT * NB], BF16, isOutput=True)

    scr = nc.dram_tensor("lnscr", [NBLK, 2 * FB], BF16)

    with tile.TileContext(nc) as tc:
        with (
            tc.tile_pool(name="const", bufs=1) as cst,
            tc.tile_pool(name="sb", bufs=1) as sb,
            tc.tile_pool(name="rzp", bufs=1, space="PSUM") as rzp,
            tc.tile_pool(name="gxp", bufs=1, space="PSUM") as gxp,
        ):
            # ---- constants (wih first: needed by the t=0 prefill) ----
            wih_sb, whh_sb = [], []
            for k in range(HT):
                w1 = cst.tile([128, 3 * H], BF16, name=f"wih_sb{k}", tag=f"wih{k}")
                nc.sync.dma_start(out=w1, in_=wih_d[k, :, :])
                wih_sb.append(w1)
            x0 = sb.tile([128, SF], BF16, name="xt0", tag="xt", bufs=3)
            nc.sync.dma_start(out=x0, in_=xs_d[0, :, :])
            for k in range(HT):
                w2 = cst.tile([128, 3 * H], BF16, name=f"whh_sb{k}", tag=f"whh{k}")
                nc.sync.dma_start(out=w2, in_=whh_d[k, :, :])
                whh_sb.append(w2)
            h0m_sb = cst.tile([128, HT * NB], BF16, name="h0m_sb", tag="h0m")
            nc.sync.dma_start(out=h0m_sb, in_=h0m_d[:, :])
            brz_sb = cst.tile([128, 8], F32, name="brz_sb", tag="brz")
            nc.sync.dma_start(out=brz_sb, in_=brz_d[:, :])
            bhn_sb = cst.tile([128, HT], F32, name="bhn_sb", tag="bhn")
            nc.sync.dma_start(out=bhn_sb, in_=bhn_d[:, :])
            bin_sb = cst.tile([128, HT], F32, name="bin_sb", tag="bin")
            nc.sync.dma_start(out=bin_sb, in_=bin_d[:, :])
            gam_sb = cst.tile([128, HT], F32, name="gam_sb", tag="gam")
            nc.sync.dma_start(out=gam_sb, in_=gam_d[:, :])
            bet_sb = cst.tile([128, HT], F32, name="bet_sb", tag="bet")
            nc.sync.dma_start(out=bet_sb, in_=bet_d[:, :])
            ind_sb = cst.tile([128, 16], BF16, name="ind_sb", tag="ind")
            nc.sync.dma_start(out=ind_sb, in_=ind_d[:, :])
            eps_sb = cst.tile([128, 1], F32, name="eps_sb", tag="eps")
            nc.vector.memset(eps_sb, float(H) * float(H) * 1e-5)

            # ---- initial (zero) state ----
            s_cur = sb.tile([128, SF], BF16, name="s_init", tag="state", bufs=3)
            nc.vector.memset(s_cur, 0.0)

            def load_x(t):
                xt = sb.tile([128, SF], BF16, name=f"xt{t}", tag="xt", bufs=3)
                nc.sync.dma_start(out=xt, in_=xs_d[t, :, :])
                return xt



            def prefill_gx(t, xt):
                # open+close gx_n groups for step t (wih only); X pair
                # [gx8|gx9], [gx10|gx11]; stays in PSUM until step t's u.
                gx_ps = [
                    gxp.tile([128, 512], F32, name=f"gx{t}_{j}", tag=f"gx{j}")
                    for j in range(2)
                ]
                for k4 in range(4):
                    j = 8 + k4
                    oap = gx_ps[k4 // 2][:, (k4 % 2) * 256 : (k4 % 2) * 256 + 256]
                    for k in range(HT):
                        nc.tensor.matmul(
                            oap, wih_sb[k][:, j * 128 : (j + 1) * 128],
                            xt[:, k * 256 : (k + 1) * 256],
                            start=(k == 0), stop=(k == HT - 1))
                return gx_ps

            xt = x0
            gx_ps = prefill_gx(0, xt)

            # LN pipeline state, keyed by block id
            ln = {}
            y_all = y2_all = None

            for t in range(T):
                main = t >= R
                toff = (t - R) % BLK
                blk = (t - R) // BLK
                blk_end = main and toff == BLK - 1

                # LN stage for an earlier block finishing this iteration
                def ln_stage(stage_t):
                    for b, st in list(ln.items()):
                        if stage_t - st["te"] in (1,):
                            yield ("s1", b, st)
                        elif stage_t - st["te"] == 2:
                            yield ("s2", b, st)
                        elif stage_t - st["te"] == 3:
                            yield ("s3", b, st)
                        elif 4 <= stage_t - st["te"] <= 7:
                            yield (f"yn{stage_t - st['te'] - 4}", b, st)

                stages = list(ln_stage(t))

                # ---- s1 (ACT top): drain the stats PSUM banks ----
                for sname, b, st in stages:
                    if sname == "s1":
                        st_mu = sb.tile([4, 512], F32, name=f"stm{b}",
                                        tag="stm", bufs=2)
                        nc.scalar.activation(
                            out=st_mu, in_=st["stp0"][0:4, :],
                            func=AF.Identity, scale=1.0)
                        st_ss = sb.tile([4, 512], F32, name=f"stv{b}",
                                        tag="stv", bufs=2)
                        nc.scalar.activation(
                            out=st_ss, in_=st["stp1"][0:4, :],
                            func=AF.Identity, scale=1.0)
                        st["st_mu"] = st_mu
                        st["st_ss"] = st_ss

                if t + 1 < T:
                    xt_nxt = load_x(t + 1)
                    mk = sb.tile([128, SF], BF16, name=f"mk{t}", tag="mask",
                                 bufs=3)
                    nc.scalar.dma_start(
                        out=mk, in_=_bcast_ap(ms_d[t + 1, :], rep=HT))

                # ---- s1 (DVE top): musq, dvar ----
                for sname, b, st in stages:
                    if sname == "s1":
                        musq = sb.tile([4, 512], F32, name=f"mq{b}", tag="mq",
                                       bufs=2)
                        nc.vector.tensor_mul(
                            musq, st["st_mu"], st["st_mu"])
                        dvar = sb.tile([4, 512], F32, name=f"dv{b}", tag="dv",
                                       bufs=2)
                        nc.vector.scalar_tensor_tensor(
                            out=dvar, in0=st["st_ss"],
                            scalar=float(H), in1=musq,
                            op0=ALU.mult, op1=ALU.subtract)
                        st["dvar"] = dvar

                # ---- s3 (DVE top): recip, pack; bounce DMAs ----
                for sname, b, st in stages:
                    if sname == "s3":
                        rst = sb.tile([4, 512], F32, name=f"rs{b}", tag="rs",
                                      bufs=2)
                        nc.vector.reciprocal_approx_fast(out=rst, in_=st["sq"])
                        pkm = sb.tile([4, 512], BF16, name=f"pkm{b}",
                                      tag="pkm", bufs=2)
                        nc.vector.tensor_copy(pkm, st["st_mu"])
                        pkr = sb.tile([4, 512], BF16, name=f"pkr{b}",
                                      tag="pkr", bufs=2)
                        nc.vector.tensor_copy(pkr, rst)
                        nc.scalar.dma_start(
                            out=scr[b : b + 1, 0:FB].rearrange(
                                "a (p f) -> (a p) f", p=4),
                            in_=pkm)
                        nc.scalar.dma_start(
                            out=scr[b : b + 1, FB : 2 * FB].rearrange(
                                "a (p f) -> (a p) f", p=4),
                            in_=pkr)
                        mu_bc = sb.tile([128, FB], BF16, name=f"mubc{b}",
                                        tag="mubc", bufs=2)
                        rs_bc = sb.tile([128, FB], BF16, name=f"rsbc{b}",
                                        tag="rsbc", bufs=2)
                        nc.scalar.dma_start(
                            out=mu_bc, in_=_bcast_ap(scr[b, 0:FB]))
                        nc.scalar.dma_start(
                            out=rs_bc, in_=_bcast_ap(scr[b, FB : 2 * FB]))
                        st["mu_bc"] = mu_bc
                        st["rs_bc"] = rs_bc

                # ---- PE: all r/z/gh groups open and close within this
                #      iteration (cross-iteration open groups on sliced
                #      tiles miscompile).  Order: independent wih opens
                #      first, state-dependent whh closes mid-stream, the
                #      complete gx prefill for t+1 last. ----
                r_ps = [
                    rzp.tile([128, 512], F32, name=f"r{t}_{j}", tag=f"r{j}")
                    for j in range(2)
                ]
                gh_ps = [
                    rzp.tile([128, 512], F32, name=f"gh{t}_{j}", tag=f"gh{j}")
                    for j in range(2)
                ]
                z_ps = [
                    rzp.tile([128, 512], F32, name=f"z{t}_{j}", tag=f"z{j}")
                    for j in range(2)
                ]

                def wih_open(ps, j4, j0):
                    # opens the j4 slice group (start zeroes the bank's
                    # write-bitmap: no other start may hit this bank until
                    # this group fully closes)
                    j = j0 + j4
                    oap = ps[j4 // 2][:, (j4 % 2) * 256 : (j4 % 2) * 256 + 256]
                    for k in range(HT):
                        nc.tensor.matmul(
                            oap, wih_sb[k][:, j * 128 : (j + 1) * 128],
                            xt[:, k * 256 : (k + 1) * 256],
                            start=(k == 0), stop=False)

                def whh_close(ps, j4, j0):
                    j = j0 + j4
                    oap = ps[j4 // 2][:, (j4 % 2) * 256 : (j4 % 2) * 256 + 256]
                    for k in range(HT):
                        nc.tensor.matmul(
                            oap, whh_sb[k][:, j * 128 : (j + 1) * 128],
                            s_cur[:, k * 256 : (k + 1) * 256],
                            start=False, stop=(k == HT - 1))

                def gh_bank(h):
                    for k4 in (2 * h, 2 * h + 1):
                        j = 8 + k4
                        oap = gh_ps[h][:, (k4 % 2) * 256 : (k4 % 2) * 256 + 256]
                        for k in range(HT):
                            nc.tensor.matmul(
                                oap, whh_sb[k][:, j * 128 : (j + 1) * 128],
                                s_cur[:, k * 256 : (k + 1) * 256],
                                start=(k == 0), stop=(k == HT - 1))

                # independent x-projections first (one open per bank),
                # then per-bank sequential close/open/close
                wih_open(r_ps, 0, 0)
                wih_open(r_ps, 2, 0)
                wih_open(z_ps, 0, 4)
                wih_open(z_ps, 2, 4)
                whh_close(r_ps, 0, 0)
                wih_open(r_ps, 1, 0)
                whh_close(r_ps, 1, 0)
                whh_close(r_ps, 2, 0)
                wih_open(r_ps, 3, 0)
                whh_close(r_ps, 3, 0)
                gh_bank(0)
                gh_bank(1)
                whh_close(z_ps, 0, 4)
                wih_open(z_ps, 1, 4)
                whh_close(z_ps, 1, 4)
                whh_close(z_ps, 2, 4)
                wih_open(z_ps, 3, 4)
                whh_close(z_ps, 3, 4)
                if t + 1 < T:
                    gx_nxt = prefill_gx(t + 1, xt_nxt)

                # ---- ACT: r sigmoids (bf16 out) ----
                r_t = sb.tile([128, SF], BF16, name=f"rt{t}", tag="rt", bufs=2)
                for k in range(HT):
                    nc.scalar.activation(
                        out=r_t[:, k * 256 : (k + 1) * 256],
                        in_=r_ps[k // 2][:, (k % 2) * 256 : (k % 2) * 256 + 256],
                        func=AF.Sigmoid, bias=brz_sb[:, k : k + 1], scale=1.0)

                # ---- DVE: w = (gh + bhn) * r  (per k-tile) ----
                w_t = sb.tile([128, SF], BF16, name=f"wt{t}", tag="wt", bufs=2)
                for k in range(HT):
                    nc.vector.scalar_tensor_tensor(
                        out=w_t[:, k * 256 : (k + 1) * 256],
                        in0=gh_ps[k // 2][:, (k % 2) * 256 : (k % 2) * 256 + 256],
                        scalar=bhn_sb[:, k : k + 1],
                        in1=r_t[:, k * 256 : (k + 1) * 256],
                        op0=ALU.add, op1=ALU.mult)

                # ---- DVE: u = w + gx (per X bank, [128,512]);
                #      Pool cannot read PSUM on TRN2 ----
                u_t = sb.tile([128, SF], BF16, name=f"ut{t}", tag="ut", bufs=2)
                for h in range(2):
                    nc.vector.tensor_add(
                        u_t[:, h * 512 : (h + 1) * 512],
                        w_t[:, h * 512 : (h + 1) * 512], gx_ps[h])

                # ---- ACT: tanh (per k-tile, bias=b_ih_n) / z sigmoids ----
                n_t = sb.tile([128, SF], BF16, name=f"nt{t}", tag="nt", bufs=2)
                z_t = sb.tile([128, SF], BF16, name=f"zt{t}", tag="zt", bufs=2)

                def tanh_k(k):
                    nc.scalar.activation(
                        out=n_t[:, k * 256 : (k + 1) * 256],
                        in_=u_t[:, k * 256 : (k + 1) * 256],
                        func=AF.Tanh, bias=bin_sb[:, k : k + 1], scale=1.0)

                def zsig_k(k):
                    nc.scalar.activation(
                        out=z_t[:, k * 256 : (k + 1) * 256],
                        in_=z_ps[k // 2][:, (k % 2) * 256 : (k % 2) * 256 + 256],
                        func=AF.Sigmoid, bias=brz_sb[:, 4 + k : 5 + k], scale=1.0)

                tanh_k(0)
                tanh_k(1)
                zsig_k(0)
                zsig_k(1)
                tanh_k(2)
                tanh_k(3)
                zsig_k(2)
                zsig_k(3)

                # ---- DVE chain (two [128,512] halves):
                #      t1 = s - n; t1 *= z; hn = t1 + n; s' = hn*mk;
                #      y = hn + x; y2 = y*y ----
                hn = sb.tile([128, SF], BF16, name=f"hn{t}", tag="hn", bufs=2)
                t1 = sb.tile([128, SF], BF16, name=f"t1{t}", tag="t1", bufs=2)
                s_nxt = None
                if t + 1 < T:
                    s_nxt = sb.tile([128, SF], BF16, name=f"s{t + 1}",
                                    tag="state", bufs=3)
                if main and toff == 0:
                    y_all = sb.tile([128, HT * FB], BF16,
                                    name=f"y{blk}", tag="y_all", bufs=2)
                    y2_all = sb.tile([128, HT * FB], BF16,
                                     name=f"y2{blk}", tag="y2_all", bufs=2)

                for h in range(2):
                    sl = slice(h * 512, (h + 1) * 512)
                    nc.vector.tensor_sub(t1[:, sl], s_cur[:, sl], n_t[:, sl])
                    nc.vector.tensor_mul(t1[:, sl], t1[:, sl], z_t[:, sl])
                    nc.vector.tensor_add(hn[:, sl], t1[:, sl], n_t[:, sl])
                    if s_nxt is not None:
                        nc.vector.tensor_mul(
                            s_nxt[:, sl], hn[:, sl], mk[:, sl])
                    if main:
                        for k in (2 * h, 2 * h + 1):
                            o = k * FB + toff * S
                            ksl = slice(k * 256, (k + 1) * 256)
                            nc.vector.tensor_add(
                                y_all[:, o : o + S], hn[:, ksl], xt[:, ksl])
                            nc.gpsimd.tensor_mul(
                                y2_all[:, o : o + S],
                                y_all[:, o : o + S], y_all[:, o : o + S])

                if debug and t == 0:
                    nc.sync.dma_start(out=dbg_d["mk0"][:, :], in_=mk)
                    nc.sync.dma_start(out=dbg_d["r0"][:, :], in_=r_t)
                    nc.sync.dma_start(out=dbg_d["z0"][:, :], in_=z_t)
                    nc.sync.dma_start(out=dbg_d["n0"][:, :], in_=n_t)
                    nc.sync.dma_start(out=dbg_d["hn0"][:, :], in_=hn)
                    nc.sync.dma_start(out=dbg_d["s1"][:, :], in_=s_nxt)
                if debug and t == 1:
                    nc.sync.dma_start(out=dbg_d["s2x"][:, :], in_=s_nxt)
                if debug and t == T - 1:
                    nc.sync.dma_start(out=dbg_d["hnT"][:, :], in_=hn)

                # -- h0 injection at entry to main (chunk-0 columns) --
                if t + 1 == R:
                    inj = s_nxt.rearrange("p (k c) -> p k c", k=HT)[:, :, 0:S:C]
                    nc.vector.tensor_copy(
                        inj, h0m_sb.rearrange("p (k n) -> p k n", k=HT))

                # -- final hidden state (chunk C-1 columns) --
                if t == T - 1:
                    hl = hn.rearrange("p (k c) -> p k c", k=HT)[
                        :, :, C - 1 : S : C]
                    nc.sync.dma_start(out=hl_d[:, :], in_=hl)

                # ---- s2 (ACT tail): sqrt (costs 2 act-table loads) ----
                for sname, b, st in stages:
                    if sname == "s2":
                        sq = sb.tile([4, 512], F32, name=f"sq{b}", tag="sq",
                                     bufs=2)
                        nc.scalar.activation(
                            out=sq, in_=st["dvar"], func=AF.Sqrt,
                            bias=eps_sb[0:4, :], scale=1.0)
                        st["sq"] = sq

                # ---- yn stages (DVE tail): normalize one k-tile ----
                for sname, b, st in stages:
                    if sname.startswith("yn"):
                        k = int(sname[2:])
                        yn = sb.tile([128, FB], BF16, name=f"yn{b}_{k}",
                                     tag="yn", bufs=3)
                        nc.vector.scalar_tensor_tensor(
                            out=yn, in0=st["y_all"][:, k * FB : (k + 1) * FB],
                            scalar=float(H), in1=st["mu_bc"],
                            op0=ALU.mult, op1=ALU.subtract)
                        nc.vector.tensor_mul(yn, yn, st["rs_bc"])
                        nc.vector.tensor_scalar(
                            out=yn, in0=yn,
                            scalar1=gam_sb[:, k : k + 1],
                            scalar2=bet_sb[:, k : k + 1],
                            op0=ALU.mult, op1=ALU.add)
                        nc.sync.dma_start(
                            out=out_d[k, :, b * FB : (b + 1) * FB], in_=yn)
                        if k == HT - 1:
                            del ln[b]

                # ---- blk_end (PE tail): LN column-sum matmuls ----
                if blk_end:
                    stp0 = rzp.tile([128, 512], F32, name=f"st{blk}a",
                                    tag="gh0")
                    stp1 = rzp.tile([128, 512], F32, name=f"st{blk}b",
                                    tag="gh1")
                    for g in range(4):   # Smu rows 0..3 <- y cols g*512..
                        for k in range(HT):
                            nc.tensor.matmul(
                                stp0[0:4, :],
                                ind_sb[:, g * 4 : (g + 1) * 4],
                                y_all[:, k * FB + g * 512 : k * FB + g * 512 + 512],
                                start=(g == 0 and k == 0),
                                stop=(g == 3 and k == HT - 1),
                                skip_group_check=True)
                    for g in range(4):   # Sss rows 0..3 <- y2
                        for k in range(HT):
                            nc.tensor.matmul(
                                stp1[0:4, :],
                                ind_sb[:, g * 4 : (g + 1) * 4],
                                y2_all[:, k * FB + g * 512 : k * FB + g * 512 + 512],
                                start=(g == 0 and k == 0),
                                stop=(g == 3 and k == HT - 1),
                                skip_group_check=True)
                    ln[blk] = {"te": t, "stp0": stp0, "stp1": stp1,
                               "y_all": y_all}

                if t + 1 < T:
                    s_cur = s_nxt
                    xt = xt_nxt
                    gx_ps = gx_nxt

            # ---- tail: finish LN for the last block(s) ----
            for b in sorted(ln):
                st = ln[b]
                st_mu = sb.tile([4, 512], F32, name=f"tstm{b}", tag="stm",
                                bufs=2)
                nc.scalar.activation(
                    out=st_mu, in_=st["stp0"][0:4, :], func=AF.Identity,
                    scale=1.0)
                st_ss = sb.tile([4, 512], F32, name=f"tstv{b}", tag="stv",
                                bufs=2)
                nc.scalar.activation(
                    out=st_ss, in_=st["stp1"][0:4, :], func=AF.Identity,
                    scale=1.0)
                musq = sb.tile([4, 512], F32, name=f"tmq{b}", tag="mq", bufs=2)
                nc.vector.tensor_mul(musq, st_mu, st_mu)
                dvar = sb.tile([4, 512], F32, name=f"tdv{b}", tag="dv", bufs=2)
                nc.vector.scalar_tensor_tensor(
                    out=dvar, in0=st_ss, scalar=float(H), in1=musq,
                    op0=ALU.mult, op1=ALU.subtract)
                sq = sb.tile([4, 512], F32, name=f"tsq{b}", tag="sq", bufs=2)
                nc.scalar.activation(
                    out=sq, in_=dvar, func=AF.Sqrt,
                    bias=eps_sb[0:4, :], scale=1.0)
                rst = sb.tile([4, 512], F32, name=f"trs{b}", tag="rs", bufs=2)
                nc.vector.reciprocal_approx_fast(out=rst, in_=sq)
                pkm = sb.tile([4, 512], BF16, name=f"tpkm{b}", tag="pkm",
                              bufs=2)
                nc.vector.tensor_copy(pkm, st_mu)
                pkr = sb.tile([4, 512], BF16, name=f"tpkr{b}", tag="pkr",
                              bufs=2)
                nc.vector.tensor_copy(pkr, rst)
                nc.scalar.dma_start(
                    out=scr[b : b + 1, 0:FB].rearrange(
                        "a (p f) -> (a p) f", p=4),
                    in_=pkm)
                nc.scalar.dma_start(
                    out=scr[b : b + 1, FB : 2 * FB].rearrange(
                        "a (p f) -> (a p) f", p=4),
                    in_=pkr)
                mu_bc = sb.tile([128, FB], BF16, name=f"tmubc{b}", tag="mubc",
                                bufs=2)
                rs_bc = sb.tile([128, FB], BF16, name=f"trsbc{b}", tag="rsbc",
                                bufs=2)
                nc.scalar.dma_start(out=mu_bc, in_=_bcast_ap(scr[b, 0:FB]))
                nc.scalar.dma_start(
                    out=rs_bc, in_=_bcast_ap(scr[b, FB : 2 * FB]))
                for k in range(HT):
                    yn = sb.tile([128, FB], BF16, name=f"tyn{b}_{k}",
                                 tag="yn", bufs=3)
                    nc.vector.scalar_tensor_tensor(
                        out=yn, in0=st["y_all"][:, k * FB : (k + 1) * FB],
                        scalar=float(H), in1=mu_bc,
                        op0=ALU.mult, op1=ALU.subtract)
                    nc.vector.tensor_mul(yn, yn, rs_bc)
                    nc.vector.tensor_scalar(
                        out=yn, in0=yn,
                        scalar1=gam_sb[:, k : k + 1],
                        scalar2=bet_sb[:, k : k + 1],
                        op0=ALU.mult, op1=ALU.add)
                    nc.sync.dma_start(
                        out=out_d[k, :, b * FB : (b + 1) * FB], in_=yn)
    nc.compile()
    return nc


def stage_inputs(input, h, is_initial, W_ih, W_hh, b_ih, b_hh, gamma, beta, R):
    """Host-side sharding/staging. Returns per-core input maps."""
    import ml_dtypes

    T = R + KS
    x = np.asarray(input, np.float32)
    h0 = np.asarray(h, np.float32)
    ii = np.asarray(is_initial).reshape(N, L)
    W_ih = np.asarray(W_ih, np.float32)
    W_hh = np.asarray(W_hh, np.float32)
    b_ih = np.asarray(b_ih, np.float32)
    b_hh = np.asarray(b_hh, np.float32)
    gamma = np.asarray(gamma, np.float32)
    beta = np.asarray(beta, np.float32)

    def bf(a):
        return np.ascontiguousarray(np.asarray(a, np.float32)).astype(
            ml_dtypes.bfloat16)

    mask = 1.0 - ii.astype(np.float32)  # [N, L]

    # l index per (c, t): warm-up reads the R steps before the chunk;
    # chunk 0's warm-up reads l in [KS-R, KS) (discarded garbage).
    l_for = np.empty((C, T), np.int64)
    for c in range(C):
        for t in range(T):
            l = c * KS + (t - R)
            l_for[c, t] = l if l >= 0 else l + KS

    wihT = np.ascontiguousarray(W_ih.T.reshape(HT, 128, 3 * H))
    whhT = np.ascontiguousarray(W_hh.T.reshape(HT, 128, 3 * H))
    brz = (b_ih + b_hh)[: 2 * H].reshape(8, 128).T.copy()        # [128, 8]
    bhn = b_hh[2 * H :].reshape(HT, 128).T.copy()                # [128, 4]
    binn = b_ih[2 * H :].reshape(HT, 128).T.copy()
    gam = gamma.reshape(HT, 128).T.copy()
    bet = beta.reshape(HT, 128).T.copy()
    # indicator stationary: group g (of 4) is a [128, 4] tile whose
    # column g is all-ones (routes a column-sum into PSUM partition g)
    ind = np.zeros((128, 16), np.float32)
    for g in range(4):
        ind[:, g * 4 + g] = 1.0

    in_maps = []
    for core in range(NCORES):
        n0 = core * NB
        xc = x[n0 : n0 + NB]              # [NB, L, H]
        xg = xc[:, l_for, :]              # [NB, C, T, H]
        # xs2[t, p, k*S + s] with s = n*C + c, h = k*128 + p
        xs2 = np.ascontiguousarray(
            xg.transpose(2, 3, 0, 1)      # [T, H, NB, C]
            .reshape(T, HT, 128, S)
            .transpose(0, 2, 1, 3)        # [T, 128, HT, S]
            .reshape(T, 128, SF))
        mg = mask[n0 : n0 + NB][:, l_for]  # [NB, C, T]
        ms = np.ascontiguousarray(mg.transpose(2, 0, 1).reshape(T, S))
        m0 = mask[n0 : n0 + NB, 0]         # [NB]
        h0c = h0[n0 : n0 + NB] * m0[:, None]     # [NB, H]
        # h0m[p, k*NB + n] = h0c[n, k*128+p]
        h0m = np.ascontiguousarray(
            h0c.reshape(NB, HT, 128).transpose(2, 1, 0).reshape(128, HT * NB))
        in_maps.append({
            "xs": bf(xs2), "ms": bf(ms), "h0m": bf(h0m),
            "wih": bf(wihT), "whh": bf(whhT),
            "brz": brz, "bhn": bhn, "bin": binn,
            "gam": gam, "bet": bet, "ind": bf(ind),
        })
    return in_maps


def required_warmup(is_initial):
    """Max distance from a chunk boundary back to the latest reset."""
    ii = np.asarray(is_initial).reshape(N, L)
    need = 0
    for c in range(1, C):
        start = c * KS
        sub = ii[:, :start]
        for n in range(N):
            nz = np.nonzero(sub[n])[0]
            gap = start - nz[-1] if len(nz) else start
            need = max(need, gap)
    return need


def unstage_outputs(results):
    out = np.empty((N, L, H), np.float32)
    h_last = np.empty((N, H), np.float32)
    for core in range(NCORES):
        n0 = core * NB
        st = np.asarray(results[core]["out_st"], np.float32)  # [HT,128,KS*S]
        o = st.reshape(HT, 128, KS, NB, C).transpose(3, 4, 2, 0, 1)
        out[n0 : n0 + NB] = o.reshape(NB, L, H)
        hl = np.asarray(results[core]["hlast"], np.float32)   # [128, HT*NB]
        h_last[n0 : n0 + NB] = (
            hl.reshape(128, HT, NB).transpose(2, 1, 0).reshape(NB, H))
    h_exp = np.broadcast_to(h_last[:, None, :], (N, L, H)).copy()
    return out, h_exp


_PROGRAM_CACHE = {}


def kernel(input, h, is_initial, W_ih, W_hh, b_ih, b_hh, gamma, beta):
    need = required_warmup(is_initial)
    R = 12
    while R < need:
        R += 4
    key = (R, True)
    if key not in _PROGRAM_CACHE:
        _PROGRAM_CACHE[key] = build_program(R)
    nc = _PROGRAM_CACHE[key]
    in_maps = stage_inputs(
        input, h, is_initial, W_ih, W_hh, b_ih, b_hh, gamma, beta, R)
    res = run_bass_kernel_spmd(nc, in_maps, list(range(NCORES))).results
    return unstage_outputs(res)


# revision 27
# speedup vs baseline: 1.2189x; 1.1424x over previous
"""Trainium2 Bass kernel for masked-GRU + residual + LayerNorm.

Problem: N=128 sequences of length L=512, hidden H=512.
  gx = x @ W_ih.T + b_ih            (precomputable input projection)
  per step l: hc = h * (1-is_initial[l]); gh = hc @ W_hh.T + b_hh
    r = sig(gx_r+gh_r); z = sig(gx_z+gh_z); n = tanh(gx_n + r*gh_n)
    h = (1-z)*n + z*hc
  out = LayerNorm(seq + x) * gamma + beta;  h_exp = broadcast(h_last)

Strategy (v2):
  * Data parallel: 16 batch rows per core (8 cores).
  * Sequence-chunk parallel: each L=512 sequence is split into C=16
    chunks of 32 steps, processed as independent columns, made exact by
    an R-step warm-up (a reset lands inside every R-window; verified at
    runtime). Chunk 0 injects the true h0 at entry to main.
  * bf16 everywhere off-PSUM: weights, x, gates, state, y, outputs.
    PSUM stays f32.  DVE runs 2-byte all-SBUF ops at 4x rate.
  * Packed state s [128, 1024] bf16 (4 h-tiles side by side in the free
    dim) so the elementwise chain runs as [128,512] halves.
  * PSUM banks (8): R pair (r gates), Z pair (z), G pair (gh_n),
    X pair (gx_n, lives across one step boundary).  LN stats borrow
    bank G0 right after it drains.
  * n-gate: gx_n stays in PSUM; w=(gh+bhh_n)*r on DVE, u=w+gx on Pool,
    tanh(u)+b_ih_n on ACT.  No identity-drain of gx.
  * LayerNorm per block of 8 steps, software-pipelined over the 7
    following steps so nothing blocks the recurrence: column sums via
    indicator-stationary matmuls into ONE [8,512] PSUM tile (rows 0-3
    Smu, rows 4-7 Sss), stats math as [4,512] multi-partition ops
    (D = H*Sss - Smu^2; rs = 1/sqrt(D + eps*H^2); out = ((H*y - Smu)
    * rs) * gamma + beta), DRAM-bounce broadcast of (Smu, rs) in bf16.
  * out / h_last stored bf16, upcast host-side.
"""
import sys

sys.path.insert(0, "/opt/trn_rl_repo")

import numpy as np

import concourse.bass as bass
import concourse.tile as tile
from concourse import bacc, mybir
from concourse.bass_utils import run_bass_kernel_spmd

F32 = mybir.dt.float32
BF16 = mybir.dt.bfloat16
AF = mybir.ActivationFunctionType
ALU = mybir.AluOpType

N, L, H = 128, 512, 512
NCORES = 8
NB = N // NCORES          # batch rows per core = 16
C = 16                    # chunks per sequence
KS = L // C               # main steps per chunk = 32
S = NB * C                # columns per core = 256
HT = H // 128             # h partition tiles = 4
BLK = 8                   # LN block (main steps)
NBLK = KS // BLK          # 4
FB = BLK * S              # 2048 block columns
SF = HT * S               # 1024 free dim of the packed state


def _bcast_ap(row_ap, parts=128, rep=1):
    """DRAM row AP -> partition-broadcast AP (0-stride over partitions,
    optionally replicated rep times along free)."""
    ap = [[0, parts]]
    if rep > 1:
        ap.append([0, rep])
    ap += [list(d) for d in row_ap.ap]
    return bass.AP(tensor=row_ap.tensor, offset=row_ap.offset, ap=ap)


def build_program(R=12, debug=False):
    T = R + KS
    nc = bacc.Bacc("TRN2", target_bir_lowering=False)

    xs_d = nc.declare_dram_parameter("xs", [T, 128, SF], BF16, isOutput=False)
    ms_d = nc.declare_dram_parameter("ms", [T, S], BF16, isOutput=False)
    h0m_d = nc.declare_dram_parameter("h0m", [128, HT * NB], BF16, isOutput=False)
    wih_d = nc.declare_dram_parameter("wih", [HT, 128, 3 * H], BF16, isOutput=False)
    whh_d = nc.declare_dram_parameter("whh", [HT, 128, 3 * H], BF16, isOutput=False)
    brz_d = nc.declare_dram_parameter("brz", [128, 8], F32, isOutput=False)
    bhn_d = nc.declare_dram_parameter("bhn", [128, HT], F32, isOutput=False)
    bin_d = nc.declare_dram_parameter("bin", [128, HT], F32, isOutput=False)
    gam_d = nc.declare_dram_parameter("gam", [128, HT], F32, isOutput=False)
    bet_d = nc.declare_dram_parameter("bet", [128, HT], F32, isOutput=False)
    ind_d = nc.declare_dram_parameter("ind", [128, 16], BF16, isOutput=False)

    out_d = nc.declare_dram_parameter("out_st", [HT, 128, KS * S], BF16, isOutput=True)
    if debug:
        dbg_names = ["mk0", "r0", "z0", "n0", "hn0", "s1", "s2x", "hnT"]
        dbg_d = {
            nm: nc.declare_dram_parameter(f"dbg_{nm}", [128, SF], BF16,
                                          isOutput=True)
            for nm in dbg_names
        }
    hl_d = nc.declare_dram_parameter("hlast", [128, HT * NB], BF16, isOutput=True)

    scr = nc.dram_tensor("lnscr", [NBLK, 8, 512], BF16)

    with tile.TileContext(nc) as tc:
        with (
            tc.tile_pool(name="const", bufs=1) as cst,
            tc.tile_pool(name="sb", bufs=1) as sb,
            tc.tile_pool(name="rzp", bufs=1, space="PSUM") as rzp,
            tc.tile_pool(name="gxp", bufs=1, space="PSUM") as gxp,
        ):
            # ---- constants (wih first: needed by the t=0 prefill) ----
            wih_sb, whh_sb = [], []
            for k in range(HT):
                w1 = cst.tile([128, 3 * H], BF16, name=f"wih_sb{k}", tag=f"wih{k}")
                nc.sync.dma_start(out=w1, in_=wih_d[k, :, :])
                wih_sb.append(w1)
            x0 = sb.tile([128, SF], BF16, name="xt0", tag="xt", bufs=3)
            nc.sync.dma_start(out=x0, in_=xs_d[0, :, :])
            for k in range(HT):
                w2 = cst.tile([128, 3 * H], BF16, name=f"whh_sb{k}", tag=f"whh{k}")
                nc.sync.dma_start(out=w2, in_=whh_d[k, :, :])
                whh_sb.append(w2)
            h0m_sb = cst.tile([128, HT * NB], BF16, name="h0m_sb", tag="h0m")
            nc.sync.dma_start(out=h0m_sb, in_=h0m_d[:, :])
            brz_sb = cst.tile([128, 8], F32, name="brz_sb", tag="brz")
            nc.sync.dma_start(out=brz_sb, in_=brz_d[:, :])
            bhn_sb = cst.tile([128, HT], F32, name="bhn_sb", tag="bhn")
            nc.sync.dma_start(out=bhn_sb, in_=bhn_d[:, :])
            bin_sb = cst.tile([128, HT], F32, name="bin_sb", tag="bin")
            nc.sync.dma_start(out=bin_sb, in_=bin_d[:, :])
            gam_sb = cst.tile([128, HT], F32, name="gam_sb", tag="gam")
            nc.sync.dma_start(out=gam_sb, in_=gam_d[:, :])
            bet_sb = cst.tile([128, HT], F32, name="bet_sb", tag="bet")
            nc.sync.dma_start(out=bet_sb, in_=bet_d[:, :])
            ind_sb = cst.tile([128, 16], BF16, name="ind_sb", tag="ind")
            nc.sync.dma_start(out=ind_sb, in_=ind_d[:, :])
            eps_sb = cst.tile([128, 1], F32, name="eps_sb", tag="eps")
            nc.vector.memset(eps_sb, float(H) * float(H) * 1e-5)

            # ---- initial (zero) state ----
            s_cur = sb.tile([128, SF], BF16, name="s_init", tag="state", bufs=3)
            nc.vector.memset(s_cur, 0.0)

            def load_x(t):
                xt = sb.tile([128, SF], BF16, name=f"xt{t}", tag="xt", bufs=3)
                nc.sync.dma_start(out=xt, in_=xs_d[t, :, :])
                return xt



            def prefill_gx(t, xt):
                # open+close gx_n groups for step t (wih only); X pair
                # [gx8|gx9], [gx10|gx11]; stays in PSUM until step t's u.
                gx_ps = [
                    gxp.tile([128, 512], F32, name=f"gx{t}_{j}", tag=f"gx{j}")
                    for j in range(2)
                ]
                for k4 in range(4):
                    j = 8 + k4
                    oap = gx_ps[k4 // 2][:, (k4 % 2) * 256 : (k4 % 2) * 256 + 256]
                    for k in range(HT):
                        nc.tensor.matmul(
                            oap, wih_sb[k][:, j * 128 : (j + 1) * 128],
                            xt[:, k * 256 : (k + 1) * 256],
                            start=(k == 0), stop=(k == HT - 1))
                return gx_ps

            xt = x0
            gx_ps = prefill_gx(0, xt)

            # LN pipeline state, keyed by block id
            ln = {}
            y_all = y2_all = None

            for t in range(T):
                main = t >= R
                toff = (t - R) % BLK
                blk = (t - R) // BLK
                blk_end = main and toff == BLK - 1

                # LN stage for an earlier block finishing this iteration
                def ln_stage(stage_t):
                    for b, st in list(ln.items()):
                        if stage_t - st["te"] in (1,):
                            yield ("s1", b, st)
                        elif stage_t - st["te"] == 2:
                            yield ("s2", b, st)
                        elif stage_t - st["te"] == 3:
                            yield ("s3", b, st)
                        elif 4 <= stage_t - st["te"] <= 7:
                            yield (f"yn{stage_t - st['te'] - 4}", b, st)

                stages = list(ln_stage(t))

                # ---- s1 (ACT top): drain the stats PSUM banks ----
                for sname, b, st in stages:
                    if sname == "s1":
                        st_mu = sb.tile([4, 512], F32, name=f"stm{b}",
                                        tag="stm", bufs=2)
                        nc.scalar.activation(
                            out=st_mu, in_=st["stp0"][0:4, :],
                            func=AF.Identity, scale=1.0)
                        st_ss = sb.tile([4, 512], F32, name=f"stv{b}",
                                        tag="stv", bufs=2)
                        nc.scalar.activation(
                            out=st_ss, in_=st["stp1"][0:4, :],
                            func=AF.Identity, scale=1.0)
                        st["st_mu"] = st_mu
                        st["st_ss"] = st_ss

                if t + 1 < T:
                    xt_nxt = load_x(t + 1)
                    mk = sb.tile([128, SF], BF16, name=f"mk{t}", tag="mask",
                                 bufs=3)
                    nc.scalar.dma_start(
                        out=mk, in_=_bcast_ap(ms_d[t + 1, :], rep=HT))

                # ---- s1 (DVE top): musq, dvar ----
                for sname, b, st in stages:
                    if sname == "s1":
                        musq = sb.tile([4, 512], F32, name=f"mq{b}", tag="mq",
                                       bufs=2)
                        nc.vector.tensor_mul(
                            musq, st["st_mu"], st["st_mu"])
                        dvar = sb.tile([4, 512], F32, name=f"dv{b}", tag="dv",
                                       bufs=2)
                        nc.vector.scalar_tensor_tensor(
                            out=dvar, in0=st["st_ss"],
                            scalar=float(H), in1=musq,
                            op0=ALU.mult, op1=ALU.subtract)
                        st["dvar"] = dvar

                # ---- s3 (DVE top): recip, pack; bounce DMAs ----
                for sname, b, st in stages:
                    if sname == "s3":
                        rst = sb.tile([4, 512], F32, name=f"rs{b}", tag="rs",
                                      bufs=2)
                        nc.vector.reciprocal_approx_fast(out=rst, in_=st["sq"])
                        pkm = sb.tile([4, 512], BF16, name=f"pkm{b}",
                                      tag="pkm", bufs=2)
                        nc.vector.tensor_copy(pkm, st["st_mu"])
                        pkr = sb.tile([4, 512], BF16, name=f"pkr{b}",
                                      tag="pkr", bufs=2)
                        nc.vector.tensor_copy(pkr, rst)
                        nc.scalar.dma_start(out=scr[b, 0:4, :], in_=pkm)
                        nc.scalar.dma_start(out=scr[b, 4:8, :], in_=pkr)
                        mu_bc = sb.tile([128, FB], BF16, name=f"mubc{b}",
                                        tag="mubc", bufs=2)
                        rs_bc = sb.tile([128, FB], BF16, name=f"rsbc{b}",
                                        tag="rsbc", bufs=2)
                        nc.scalar.dma_start(
                            out=mu_bc,
                            in_=_bcast_ap(scr[b, 0:4, :].rearrange("p f -> (p f)")))
                        nc.scalar.dma_start(
                            out=rs_bc,
                            in_=_bcast_ap(scr[b, 4:8, :].rearrange("p f -> (p f)")))
                        st["mu_bc"] = mu_bc
                        st["rs_bc"] = rs_bc

                # ---- PE: all r/z/gh groups open and close within this
                #      iteration (cross-iteration open groups on sliced
                #      tiles miscompile).  Order: independent wih opens
                #      first, state-dependent whh closes mid-stream, the
                #      complete gx prefill for t+1 last. ----
                r_ps = [
                    rzp.tile([128, 512], F32, name=f"r{t}_{j}", tag=f"r{j}")
                    for j in range(2)
                ]
                gh_ps = [
                    rzp.tile([128, 512], F32, name=f"gh{t}_{j}", tag=f"gh{j}")
                    for j in range(2)
                ]
                z_ps = [
                    rzp.tile([128, 512], F32, name=f"z{t}_{j}", tag=f"z{j}")
                    for j in range(2)
                ]

                def wih_open(ps, j4, j0):
                    # opens the j4 slice group (start zeroes the bank's
                    # write-bitmap: no other start may hit this bank until
                    # this group fully closes)
                    j = j0 + j4
                    oap = ps[j4 // 2][:, (j4 % 2) * 256 : (j4 % 2) * 256 + 256]
                    for k in range(HT):
                        nc.tensor.matmul(
                            oap, wih_sb[k][:, j * 128 : (j + 1) * 128],
                            xt[:, k * 256 : (k + 1) * 256],
                            start=(k == 0), stop=False)

                def whh_close(ps, j4, j0):
                    j = j0 + j4
                    oap = ps[j4 // 2][:, (j4 % 2) * 256 : (j4 % 2) * 256 + 256]
                    for k in range(HT):
                        nc.tensor.matmul(
                            oap, whh_sb[k][:, j * 128 : (j + 1) * 128],
                            s_cur[:, k * 256 : (k + 1) * 256],
                            start=False, stop=(k == HT - 1))

                def gh_bank(h):
                    for k4 in (2 * h, 2 * h + 1):
                        j = 8 + k4
                        oap = gh_ps[h][:, (k4 % 2) * 256 : (k4 % 2) * 256 + 256]
                        for k in range(HT):
                            nc.tensor.matmul(
                                oap, whh_sb[k][:, j * 128 : (j + 1) * 128],
                                s_cur[:, k * 256 : (k + 1) * 256],
                                start=(k == 0), stop=(k == HT - 1))

                # independent x-projections first (one open per bank),
                # then per-bank sequential close/open/close
                wih_open(r_ps, 0, 0)
                wih_open(r_ps, 2, 0)
                wih_open(z_ps, 0, 4)
                wih_open(z_ps, 2, 4)
                whh_close(r_ps, 0, 0)
                wih_open(r_ps, 1, 0)
                whh_close(r_ps, 1, 0)
                whh_close(r_ps, 2, 0)
                wih_open(r_ps, 3, 0)
                whh_close(r_ps, 3, 0)
                gh_bank(0)
                gh_bank(1)
                whh_close(z_ps, 0, 4)
                wih_open(z_ps, 1, 4)
                whh_close(z_ps, 1, 4)
                whh_close(z_ps, 2, 4)
                wih_open(z_ps, 3, 4)
                whh_close(z_ps, 3, 4)
                if t + 1 < T:
                    gx_nxt = prefill_gx(t + 1, xt_nxt)

                # ---- ACT: r sigmoids (bf16 out) ----
                r_t = sb.tile([128, SF], BF16, name=f"rt{t}", tag="rt", bufs=2)
                for k in range(HT):
                    nc.scalar.activation(
                        out=r_t[:, k * 256 : (k + 1) * 256],
                        in_=r_ps[k // 2][:, (k % 2) * 256 : (k % 2) * 256 + 256],
                        func=AF.Sigmoid, bias=brz_sb[:, k : k + 1], scale=1.0)

                # ---- DVE: w = (gh + bhn) * r  (per k-tile) ----
                w_t = sb.tile([128, SF], BF16, name=f"wt{t}", tag="wt", bufs=2)
                for k in range(HT):
                    nc.vector.scalar_tensor_tensor(
                        out=w_t[:, k * 256 : (k + 1) * 256],
                        in0=gh_ps[k // 2][:, (k % 2) * 256 : (k % 2) * 256 + 256],
                        scalar=bhn_sb[:, k : k + 1],
                        in1=r_t[:, k * 256 : (k + 1) * 256],
                        op0=ALU.add, op1=ALU.mult)

                # ---- DVE: u = w + gx (per X bank, [128,512]);
                #      Pool cannot read PSUM on TRN2 ----
                u_t = sb.tile([128, SF], BF16, name=f"ut{t}", tag="ut", bufs=2)
                for h in range(2):
                    nc.vector.tensor_add(
                        u_t[:, h * 512 : (h + 1) * 512],
                        w_t[:, h * 512 : (h + 1) * 512], gx_ps[h])

                # ---- ACT: tanh (per k-tile, bias=b_ih_n) / z sigmoids ----
                n_t = sb.tile([128, SF], BF16, name=f"nt{t}", tag="nt", bufs=2)
                z_t = sb.tile([128, SF], BF16, name=f"zt{t}", tag="zt", bufs=2)

                def tanh_k(k):
                    nc.scalar.activation(
                        out=n_t[:, k * 256 : (k + 1) * 256],
                        in_=u_t[:, k * 256 : (k + 1) * 256],
                        func=AF.Tanh, bias=bin_sb[:, k : k + 1], scale=1.0)

                def zsig_k(k):
                    nc.scalar.activation(
                        out=z_t[:, k * 256 : (k + 1) * 256],
                        in_=z_ps[k // 2][:, (k % 2) * 256 : (k % 2) * 256 + 256],
                        func=AF.Sigmoid, bias=brz_sb[:, 4 + k : 5 + k], scale=1.0)

                tanh_k(0)
                tanh_k(1)
                zsig_k(0)
                zsig_k(1)
                tanh_k(2)
                tanh_k(3)
                zsig_k(2)
                zsig_k(3)

                # ---- DVE chain (two [128,512] halves):
                #      t1 = s - n; t1 *= z; hn = t1 + n; s' = hn*mk;
                #      y = hn + x; y2 = y*y ----
                hn = sb.tile([128, SF], BF16, name=f"hn{t}", tag="hn", bufs=2)
                t1 = sb.tile([128, SF], BF16, name=f"t1{t}", tag="t1", bufs=2)
                s_nxt = None
                if t + 1 < T:
                    s_nxt = sb.tile([128, SF], BF16, name=f"s{t + 1}",
                                    tag="state", bufs=3)
                if main and toff == 0:
                    y_all = sb.tile([128, HT * FB], BF16,
                                    name=f"y{blk}", tag="y_all", bufs=2)
                    y2_all = sb.tile([128, HT * FB], BF16,
                                     name=f"y2{blk}", tag="y2_all", bufs=2)

                for h in range(2):
                    sl = slice(h * 512, (h + 1) * 512)
                    nc.vector.tensor_sub(t1[:, sl], s_cur[:, sl], n_t[:, sl])
                    nc.vector.tensor_mul(t1[:, sl], t1[:, sl], z_t[:, sl])
                    nc.vector.tensor_add(hn[:, sl], t1[:, sl], n_t[:, sl])
                    if s_nxt is not None:
                        nc.vector.tensor_mul(
                            s_nxt[:, sl], hn[:, sl], mk[:, sl])
                    if main:
                        for k in (2 * h, 2 * h + 1):
                            o = k * FB + toff * S
                            ksl = slice(k * 256, (k + 1) * 256)
                            nc.vector.tensor_add(
                                y_all[:, o : o + S], hn[:, ksl], xt[:, ksl])
                            nc.gpsimd.tensor_mul(
                                y2_all[:, o : o + S],
                                y_all[:, o : o + S], y_all[:, o : o + S])

                if debug and t == 0:
                    nc.sync.dma_start(out=dbg_d["mk0"][:, :], in_=mk)
                    nc.sync.dma_start(out=dbg_d["r0"][:, :], in_=r_t)
                    nc.sync.dma_start(out=dbg_d["z0"][:, :], in_=z_t)
                    nc.sync.dma_start(out=dbg_d["n0"][:, :], in_=n_t)
                    nc.sync.dma_start(out=dbg_d["hn0"][:, :], in_=hn)
                    nc.sync.dma_start(out=dbg_d["s1"][:, :], in_=s_nxt)
                if debug and t == 1:
                    nc.sync.dma_start(out=dbg_d["s2x"][:, :], in_=s_nxt)
                if debug and t == T - 1:
                    nc.sync.dma_start(out=dbg_d["hnT"][:, :], in_=hn)

                # -- h0 injection at entry to main (chunk-0 columns) --
                if t + 1 == R:
                    inj = s_nxt.rearrange("p (k c) -> p k c", k=HT)[:, :, 0:S:C]
                    nc.vector.tensor_copy(
                        inj, h0m_sb.rearrange("p (k n) -> p k n", k=HT))

                # -- final hidden state (chunk C-1 columns) --
                if t == T - 1:
                    hl = hn.rearrange("p (k c) -> p k c", k=HT)[
                        :, :, C - 1 : S : C]
                    nc.sync.dma_start(out=hl_d[:, :], in_=hl)

                # ---- s2 (ACT tail): sqrt (costs 2 act-table loads) ----
                for sname, b, st in stages:
                    if sname == "s2":
                        sq = sb.tile([4, 512], F32, name=f"sq{b}", tag="sq",
                                     bufs=2)
                        nc.scalar.activation(
                            out=sq, in_=st["dvar"], func=AF.Sqrt,
                            bias=eps_sb[0:4, :], scale=1.0)
                        st["sq"] = sq

                # ---- yn stages (DVE tail): normalize one k-tile ----
                for sname, b, st in stages:
                    if sname.startswith("yn"):
                        k = int(sname[2:])
                        yn = sb.tile([128, FB], BF16, name=f"yn{b}_{k}",
                                     tag="yn", bufs=3)
                        nc.vector.scalar_tensor_tensor(
                            out=yn, in0=st["y_all"][:, k * FB : (k + 1) * FB],
                            scalar=float(H), in1=st["mu_bc"],
                            op0=ALU.mult, op1=ALU.subtract)
                        nc.vector.tensor_mul(yn, yn, st["rs_bc"])
                        nc.vector.tensor_scalar(
                            out=yn, in0=yn,
                            scalar1=gam_sb[:, k : k + 1],
                            scalar2=bet_sb[:, k : k + 1],
                            op0=ALU.mult, op1=ALU.add)
                        nc.sync.dma_start(
                            out=out_d[k, :, b * FB : (b + 1) * FB], in_=yn)
                        if k == HT - 1:
                            del ln[b]

                # ---- blk_end (PE tail): LN column-sum matmuls ----
                if blk_end:
                    stp0 = rzp.tile([128, 512], F32, name=f"st{blk}a",
                                    tag="gh0")
                    stp1 = rzp.tile([128, 512], F32, name=f"st{blk}b",
                                    tag="gh1")
                    for g in range(4):   # Smu rows 0..3 <- y cols g*512..
                        for k in range(HT):
                            nc.tensor.matmul(
                                stp0[0:4, :],
                                ind_sb[:, g * 4 : (g + 1) * 4],
                                y_all[:, k * FB + g * 512 : k * FB + g * 512 + 512],
                                start=(g == 0 and k == 0),
                                stop=(g == 3 and k == HT - 1),
                                skip_group_check=True)
                    for g in range(4):   # Sss rows 0..3 <- y2
                        for k in range(HT):
                            nc.tensor.matmul(
                                stp1[0:4, :],
                                ind_sb[:, g * 4 : (g + 1) * 4],
                                y2_all[:, k * FB + g * 512 : k * FB + g * 512 + 512],
                                start=(g == 0 and k == 0),
                                stop=(g == 3 and k == HT - 1),
                                skip_group_check=True)
                    ln[blk] = {"te": t, "stp0": stp0, "stp1": stp1,
                               "y_all": y_all}

                if t + 1 < T:
                    s_cur = s_nxt
                    xt = xt_nxt
                    gx_ps = gx_nxt

            # ---- tail: finish LN for the last block(s) ----
            for b in sorted(ln):
                st = ln[b]
                st_mu = sb.tile([4, 512], F32, name=f"tstm{b}", tag="stm",
                                bufs=2)
                nc.scalar.activation(
                    out=st_mu, in_=st["stp0"][0:4, :], func=AF.Identity,
                    scale=1.0)
                st_ss = sb.tile([4, 512], F32, name=f"tstv{b}", tag="stv",
                                bufs=2)
                nc.scalar.activation(
                    out=st_ss, in_=st["stp1"][0:4, :], func=AF.Identity,
                    scale=1.0)
                musq = sb.tile([4, 512], F32, name=f"tmq{b}", tag="mq", bufs=2)
                nc.vector.tensor_mul(musq, st_mu, st_mu)
                dvar = sb.tile([4, 512], F32, name=f"tdv{b}", tag="dv", bufs=2)
                nc.vector.scalar_tensor_tensor(
                    out=dvar, in0=st_ss, scalar=float(H), in1=musq,
                    op0=ALU.mult, op1=ALU.subtract)
                sq = sb.tile([4, 512], F32, name=f"tsq{b}", tag="sq", bufs=2)
                nc.scalar.activation(
                    out=sq, in_=dvar, func=AF.Sqrt,
                    bias=eps_sb[0:4, :], scale=1.0)
                rst = sb.tile([4, 512], F32, name=f"trs{b}", tag="rs", bufs=2)
                nc.vector.reciprocal_approx_fast(out=rst, in_=sq)
                pkm = sb.tile([4, 512], BF16, name=f"tpkm{b}", tag="pkm",
                              bufs=2)
                nc.vector.tensor_copy(pkm, st_mu)
                pkr = sb.tile([4, 512], BF16, name=f"tpkr{b}", tag="pkr",
                              bufs=2)
                nc.vector.tensor_copy(pkr, rst)
                nc.scalar.dma_start(out=scr[b, 0:4, :], in_=pkm)
                nc.scalar.dma_start(out=scr[b, 4:8, :], in_=pkr)
                mu_bc = sb.tile([128, FB], BF16, name=f"tmubc{b}", tag="mubc",
                                bufs=2)
                rs_bc = sb.tile([128, FB], BF16, name=f"trsbc{b}", tag="rsbc",
                                bufs=2)
                nc.scalar.dma_start(
                    out=mu_bc,
                    in_=_bcast_ap(scr[b, 0:4, :].rearrange("p f -> (p f)")))
                nc.scalar.dma_start(
                    out=rs_bc,
                    in_=_bcast_ap(scr[b, 4:8, :].rearrange("p f -> (p f)")))
                for k in range(HT):
                    yn = sb.tile([128, FB], BF16, name=f"tyn{b}_{k}",
                                 tag="yn", bufs=3)
                    nc.vector.scalar_tensor_tensor(
                        out=yn, in0=st["y_all"][:, k * FB : (k + 1) * FB],
                        scalar=float(H), in1=mu_bc,
                        op0=ALU.mult, op1=ALU.subtract)
                    nc.vector.tensor_mul(yn, yn, rs_bc)
                    nc.vector.tensor_scalar(
                        out=yn, in0=yn,
                        scalar1=gam_sb[:, k : k + 1],
                        scalar2=bet_sb[:, k : k + 1],
                        op0=ALU.mult, op1=ALU.add)
                    nc.sync.dma_start(
                        out=out_d[k, :, b * FB : (b + 1) * FB], in_=yn)
    nc.compile()
    return nc


def stage_inputs(input, h, is_initial, W_ih, W_hh, b_ih, b_hh, gamma, beta, R):
    """Host-side sharding/staging. Returns per-core input maps."""
    import ml_dtypes

    T = R + KS
    x = np.asarray(input, np.float32)
    h0 = np.asarray(h, np.float32)
    ii = np.asarray(is_initial).reshape(N, L)
    W_ih = np.asarray(W_ih, np.float32)
    W_hh = np.asarray(W_hh, np.float32)
    b_ih = np.asarray(b_ih, np.float32)
    b_hh = np.asarray(b_hh, np.float32)
    gamma = np.asarray(gamma, np.float32)
    beta = np.asarray(beta, np.float32)

    def bf(a):
        return np.ascontiguousarray(np.asarray(a, np.float32)).astype(
            ml_dtypes.bfloat16)

    mask = 1.0 - ii.astype(np.float32)  # [N, L]

    # l index per (c, t): warm-up reads the R steps before the chunk;
    # chunk 0's warm-up reads l in [KS-R, KS) (discarded garbage).
    l_for = np.empty((C, T), np.int64)
    for c in range(C):
        for t in range(T):
            l = c * KS + (t - R)
            l_for[c, t] = l if l >= 0 else l + KS

    wihT = np.ascontiguousarray(W_ih.T.reshape(HT, 128, 3 * H))
    whhT = np.ascontiguousarray(W_hh.T.reshape(HT, 128, 3 * H))
    brz = (b_ih + b_hh)[: 2 * H].reshape(8, 128).T.copy()        # [128, 8]
    bhn = b_hh[2 * H :].reshape(HT, 128).T.copy()                # [128, 4]
    binn = b_ih[2 * H :].reshape(HT, 128).T.copy()
    gam = gamma.reshape(HT, 128).T.copy()
    bet = beta.reshape(HT, 128).T.copy()
    # indicator stationary: group g (of 4) is a [128, 4] tile whose
    # column g is all-ones (routes a column-sum into PSUM partition g)
    ind = np.zeros((128, 16), np.float32)
    for g in range(4):
        ind[:, g * 4 + g] = 1.0

    in_maps = []
    for core in range(NCORES):
        n0 = core * NB
        xc = x[n0 : n0 + NB]              # [NB, L, H]
        xg = xc[:, l_for, :]              # [NB, C, T, H]
        # xs2[t, p, k*S + s] with s = n*C + c, h = k*128 + p
        xs2 = np.ascontiguousarray(
            xg.transpose(2, 3, 0, 1)      # [T, H, NB, C]
            .reshape(T, HT, 128, S)
            .transpose(0, 2, 1, 3)        # [T, 128, HT, S]
            .reshape(T, 128, SF))
        mg = mask[n0 : n0 + NB][:, l_for]  # [NB, C, T]
        ms = np.ascontiguousarray(mg.transpose(2, 0, 1).reshape(T, S))
        m0 = mask[n0 : n0 + NB, 0]         # [NB]
        h0c = h0[n0 : n0 + NB] * m0[:, None]     # [NB, H]
        # h0m[p, k*NB + n] = h0c[n, k*128+p]
        h0m = np.ascontiguousarray(
            h0c.reshape(NB, HT, 128).transpose(2, 1, 0).reshape(128, HT * NB))
        in_maps.append({
            "xs": bf(xs2), "ms": bf(ms), "h0m": bf(h0m),
            "wih": bf(wihT), "whh": bf(whhT),
            "brz": brz, "bhn": bhn, "bin": binn,
            "gam": gam, "bet": bet, "ind": bf(ind),
        })
    return in_maps


def required_warmup(is_initial):
    """Max distance from a chunk boundary back to the latest reset."""
    ii = np.asarray(is_initial).reshape(N, L)
    need = 0
    for c in range(1, C):
        start = c * KS
        sub = ii[:, :start]
        for n in range(N):
            nz = np.nonzero(sub[n])[0]
            gap = start - nz[-1] if len(nz) else start
            need = max(need, gap)
    return need


def unstage_outputs(results):
    out = np.empty((N, L, H), np.float32)
    h_last = np.empty((N, H), np.float32)
    for core in range(NCORES):
        n0 = core * NB
        st = np.asarray(results[core]["out_st"], np.float32)  # [HT,128,KS*S]
        o = st.reshape(HT, 128, KS, NB, C).transpose(3, 4, 2, 0, 1)
        out[n0 : n0 + NB] = o.reshape(NB, L, H)
        hl = np.asarray(results[core]["hlast"], np.float32)   # [128, HT*NB]
        h_last[n0 : n0 + NB] = (
            hl.reshape(128, HT, NB).transpose(2, 1, 0).reshape(NB, H))
    h_exp = np.broadcast_to(h_last[:, None, :], (N, L, H)).copy()
    return out, h_exp


_PROGRAM_CACHE = {}


def kernel(input, h, is_initial, W_ih, W_hh, b_ih, b_hh, gamma, beta):
    need = required_warmup(is_initial)
    R = 12
    while R < need:
        R += 4
    key = (R, True)
    if key not in _PROGRAM_CACHE:
        _PROGRAM_CACHE[key] = build_program(R)
    nc = _PROGRAM_CACHE[key]
    in_maps = stage_inputs(
        input, h, is_initial, W_ih, W_hh, b_ih, b_hh, gamma, beta, R)
    res = run_bass_kernel_spmd(nc, in_maps, list(range(NCORES))).results
    return unstage_outputs(res)


# revision 31
# speedup vs baseline: 1.3101x; 1.0748x over previous
"""Trainium2 Bass kernel for masked-GRU + residual + LayerNorm.

Problem: N=128 sequences of length L=512, hidden H=512.
  gx = x @ W_ih.T + b_ih            (precomputable input projection)
  per step l: hc = h * (1-is_initial[l]); gh = hc @ W_hh.T + b_hh
    r = sig(gx_r+gh_r); z = sig(gx_z+gh_z); n = tanh(gx_n + r*gh_n)
    h = (1-z)*n + z*hc
  out = LayerNorm(seq + x) * gamma + beta;  h_exp = broadcast(h_last)

Strategy (v2):
  * Data parallel: 16 batch rows per core (8 cores).
  * Sequence-chunk parallel: each L=512 sequence is split into C=16
    chunks of 32 steps, processed as independent columns, made exact by
    an R-step warm-up (a reset lands inside every R-window; verified at
    runtime). Chunk 0 injects the true h0 at entry to main.
  * bf16 everywhere off-PSUM: weights, x, gates, state, y, outputs.
    PSUM stays f32.  DVE runs 2-byte all-SBUF ops at 4x rate.
  * Packed state s [128, 1024] bf16 (4 h-tiles side by side in the free
    dim) so the elementwise chain runs as [128,512] halves.
  * PSUM banks (8): R pair (r gates), Z pair (z), G pair (gh_n),
    X pair (gx_n, lives across one step boundary).  LN stats borrow
    bank G0 right after it drains.
  * n-gate: gx_n stays in PSUM; w=(gh+bhh_n)*r on DVE, u=w+gx on Pool,
    tanh(u)+b_ih_n on ACT.  No identity-drain of gx.
  * LayerNorm per block of 8 steps, software-pipelined over the 7
    following steps so nothing blocks the recurrence: column sums via
    indicator-stationary matmuls into ONE [8,512] PSUM tile (rows 0-3
    Smu, rows 4-7 Sss), stats math as [4,512] multi-partition ops
    (D = H*Sss - Smu^2; rs = 1/sqrt(D + eps*H^2); out = ((H*y - Smu)
    * rs) * gamma + beta), DRAM-bounce broadcast of (Smu, rs) in bf16.
  * out / h_last stored bf16, upcast host-side.
"""
import sys

sys.path.insert(0, "/opt/trn_rl_repo")

import numpy as np

import concourse.bass as bass
import concourse.tile as tile
from concourse import bacc, mybir
from concourse.bass_utils import run_bass_kernel_spmd

F32 = mybir.dt.float32
BF16 = mybir.dt.bfloat16
AF = mybir.ActivationFunctionType
ALU = mybir.AluOpType

N, L, H = 128, 512, 512
NCORES = 8
NB = N // NCORES          # batch rows per core = 16
C = 16                    # chunks per sequence
KS = L // C               # main steps per chunk = 32
S = NB * C                # columns per core = 256
HT = H // 128             # h partition tiles = 4
BLK = 8                   # LN block (main steps)
NBLK = KS // BLK          # 4
FB = BLK * S              # 2048 block columns
SF = HT * S               # 1024 free dim of the packed state


def _bcast_ap(row_ap, parts=128, rep=1):
    """DRAM row AP -> partition-broadcast AP (0-stride over partitions,
    optionally replicated rep times along free)."""
    ap = [[0, parts]]
    if rep > 1:
        ap.append([0, rep])
    ap += [list(d) for d in row_ap.ap]
    return bass.AP(tensor=row_ap.tensor, offset=row_ap.offset, ap=ap)


def build_program(R=12, debug=False):
    T = R + KS
    nc = bacc.Bacc("TRN2", target_bir_lowering=False)

    xs_d = nc.declare_dram_parameter("xs", [T, 128, SF], BF16, isOutput=False)
    ms_d = nc.declare_dram_parameter("ms", [T, S], BF16, isOutput=False)
    h0m_d = nc.declare_dram_parameter("h0m", [128, HT * NB], BF16, isOutput=False)
    wih_d = nc.declare_dram_parameter("wih", [HT, 128, 3 * H], BF16, isOutput=False)
    whh_d = nc.declare_dram_parameter("whh", [HT, 128, 3 * H], BF16, isOutput=False)
    brz_d = nc.declare_dram_parameter("brz", [128, 8], F32, isOutput=False)
    bhn_d = nc.declare_dram_parameter("bhn", [128, HT], F32, isOutput=False)
    bin_d = nc.declare_dram_parameter("bin", [128, HT], F32, isOutput=False)
    gam_d = nc.declare_dram_parameter("gam", [128, HT], F32, isOutput=False)
    bet_d = nc.declare_dram_parameter("bet", [128, HT], F32, isOutput=False)
    ind_d = nc.declare_dram_parameter("ind", [128, 16], BF16, isOutput=False)
    sel_d = nc.declare_dram_parameter("sel", [4, 512], BF16, isOutput=False)

    out_d = nc.declare_dram_parameter("out_st", [HT, 128, KS * S], BF16, isOutput=True)
    if debug:
        dbg_names = ["mk0", "r0", "z0", "n0", "hn0", "s1", "s2x", "hnT"]
        dbg_d = {
            nm: nc.declare_dram_parameter(f"dbg_{nm}", [128, SF], BF16,
                                          isOutput=True)
            for nm in dbg_names
        }
    hl_d = nc.declare_dram_parameter("hlast", [128, HT * NB], BF16, isOutput=True)


    with tile.TileContext(nc) as tc:
        with (
            tc.tile_pool(name="const", bufs=1) as cst,
            tc.tile_pool(name="sb", bufs=1) as sb,
            tc.tile_pool(name="rzp", bufs=1, space="PSUM") as rzp,
            tc.tile_pool(name="gxp", bufs=1, space="PSUM") as gxp,
        ):
            # ---- constants (wih first: needed by the t=0 prefill) ----
            wih_sb, whh_sb = [], []
            for k in range(HT):
                w1 = cst.tile([128, 3 * H], BF16, name=f"wih_sb{k}", tag=f"wih{k}")
                nc.sync.dma_start(out=w1, in_=wih_d[k, :, :])
                wih_sb.append(w1)
            x0 = sb.tile([128, SF], BF16, name="xt0", tag="xt", bufs=3)
            nc.sync.dma_start(out=x0, in_=xs_d[0, :, :])
            for k in range(HT):
                w2 = cst.tile([128, 3 * H], BF16, name=f"whh_sb{k}", tag=f"whh{k}")
                nc.sync.dma_start(out=w2, in_=whh_d[k, :, :])
                whh_sb.append(w2)
            h0m_sb = cst.tile([128, HT * NB], BF16, name="h0m_sb", tag="h0m")
            nc.sync.dma_start(out=h0m_sb, in_=h0m_d[:, :])
            brz_sb = cst.tile([128, 8], F32, name="brz_sb", tag="brz")
            nc.sync.dma_start(out=brz_sb, in_=brz_d[:, :])
            bhn_sb = cst.tile([128, HT], F32, name="bhn_sb", tag="bhn")
            nc.sync.dma_start(out=bhn_sb, in_=bhn_d[:, :])
            bin_sb = cst.tile([128, HT], F32, name="bin_sb", tag="bin")
            nc.sync.dma_start(out=bin_sb, in_=bin_d[:, :])
            gam_sb = cst.tile([128, HT], F32, name="gam_sb", tag="gam")
            nc.sync.dma_start(out=gam_sb, in_=gam_d[:, :])
            bet_sb = cst.tile([128, HT], F32, name="bet_sb", tag="bet")
            nc.sync.dma_start(out=bet_sb, in_=bet_d[:, :])
            ind_sb = cst.tile([128, 16], BF16, name="ind_sb", tag="ind")
            nc.sync.dma_start(out=ind_sb, in_=ind_d[:, :])
            eps_sb = cst.tile([128, 1], F32, name="eps_sb", tag="eps")
            nc.vector.memset(eps_sb, float(H) * float(H) * 1e-5)
            sel_sb = cst.tile([4, 512], BF16, name="sel_sb", tag="sel")
            nc.sync.dma_start(out=sel_sb, in_=sel_d[:, :])
            # masks: load once to partition 0, broadcast on Pool
            ms_all = cst.tile([128, T * S], BF16, name="ms_all", tag="msb")
            nc.sync.dma_start(
                out=ms_all[0:1, :], in_=ms_d[:, :].rearrange("t s -> (t s)"))
            nc.gpsimd.partition_broadcast(ms_all, ms_all[0:1, :])

            # ---- initial (zero) state ----
            s_cur = sb.tile([128, SF], BF16, name="s_init", tag="state", bufs=3)
            nc.vector.memset(s_cur, 0.0)

            def load_x(t):
                xt = sb.tile([128, SF], BF16, name=f"xt{t}", tag="xt", bufs=3)
                nc.sync.dma_start(out=xt, in_=xs_d[t, :, :])
                return xt



            def prefill_gx(t, xt):
                # open+close gx_n groups for step t (wih only); X pair
                # [gx8|gx9], [gx10|gx11]; stays in PSUM until step t's u.
                gx_ps = [
                    gxp.tile([128, 512], F32, name=f"gx{t}_{j}", tag=f"gx{j}")
                    for j in range(2)
                ]
                for k4 in range(4):
                    j = 8 + k4
                    oap = gx_ps[k4 // 2][:, (k4 % 2) * 256 : (k4 % 2) * 256 + 256]
                    for k in range(HT):
                        nc.tensor.matmul(
                            oap, wih_sb[k][:, j * 128 : (j + 1) * 128],
                            xt[:, k * 256 : (k + 1) * 256],
                            start=(k == 0), stop=(k == HT - 1))
                return gx_ps

            xt = x0
            gx_ps = prefill_gx(0, xt)

            # LN pipeline state, keyed by block id
            ln = {}
            y_all = y2_all = None

            for t in range(T):
                main = t >= R
                toff = (t - R) % BLK
                blk = (t - R) // BLK
                blk_end = main and toff == BLK - 1

                # LN pipeline stage for earlier blocks this iteration
                stages = [(t - st["te"], b, st) for b, st in list(ln.items())
                          if 1 <= t - st["te"] <= 8]

                # ---- s1 (ACT top): drain the stats PSUM banks ----
                for dt_, b, st in stages:
                    if dt_ == 1:
                        st_mu = sb.tile([4, 512], F32, name=f"stm{b}",
                                        tag="stm", bufs=2)
                        nc.scalar.activation(
                            out=st_mu, in_=st["stp0"][0:4, :],
                            func=AF.Identity, scale=1.0)
                        st_ss = sb.tile([4, 512], F32, name=f"stv{b}",
                                        tag="stv", bufs=2)
                        nc.scalar.activation(
                            out=st_ss, in_=st["stp1"][0:4, :],
                            func=AF.Identity, scale=1.0)
                        st["st_mu"] = st_mu
                        st["st_ss"] = st_ss

                if t + 1 < T:
                    xt_nxt = load_x(t + 1)
                    mk = ms_all[:, (t + 1) * S : (t + 2) * S]

                # ---- s1 (DVE top): musq, dvar ----
                for dt_, b, st in stages:
                    if dt_ == 1:
                        musq = sb.tile([4, 512], F32, name=f"mq{b}", tag="mq",
                                       bufs=2)
                        nc.vector.tensor_mul(
                            musq, st["st_mu"], st["st_mu"])
                        dvar = sb.tile([4, 512], F32, name=f"dv{b}", tag="dv",
                                       bufs=2)
                        nc.vector.scalar_tensor_tensor(
                            out=dvar, in0=st["st_ss"],
                            scalar=float(H), in1=musq,
                            op0=ALU.mult, op1=ALU.subtract)
                        st["dvar"] = dvar

                # ---- s3 (DVE top): recip + bf16 packs ----
                for dt_, b, st in stages:
                    if dt_ == 3:
                        rst = sb.tile([4, 512], F32, name=f"rs{b}", tag="rs",
                                      bufs=2)
                        nc.vector.reciprocal_approx_fast(out=rst, in_=st["sq"])
                        pkm = sb.tile([4, 512], BF16, name=f"pkm{b}",
                                      tag="pkm", bufs=2)
                        nc.vector.tensor_copy(pkm, st["st_mu"])
                        pkr = sb.tile([4, 512], BF16, name=f"pkr{b}",
                                      tag="pkr", bufs=2)
                        nc.vector.tensor_copy(pkr, rst)
                        st["pkm"] = pkm
                        st["pkr"] = pkr
                        st["mu_bc"] = sb.tile([128, FB], BF16, name=f"mubc{b}",
                                              tag="mubc", bufs=1)
                        st["rs_bc"] = sb.tile([128, FB], BF16, name=f"rsbc{b}",
                                              tag="rsbc", bufs=1)

                # ---- s4/s5 (ACT top): drain bc PSUM waves to SBUF ----
                for dt_, b, st in stages:
                    if dt_ in (4, 5):
                        for g, mu_ps, rs_ps in st.get(f"bcw{dt_}", []):
                            nc.scalar.activation(
                                out=st["mu_bc"][:, g * 512 : (g + 1) * 512],
                                in_=mu_ps, func=AF.Identity, scale=1.0)
                            nc.scalar.activation(
                                out=st["rs_bc"][:, g * 512 : (g + 1) * 512],
                                in_=rs_ps, func=AF.Identity, scale=1.0)

                # ---- PE: all r/z/gh groups open and close within this
                #      iteration (cross-iteration open groups on sliced
                #      tiles miscompile).  Order: independent wih opens
                #      first, state-dependent whh closes mid-stream, the
                #      complete gx prefill for t+1 last. ----
                r_ps = [
                    rzp.tile([128, 512], F32, name=f"r{t}_{j}", tag=f"r{j}")
                    for j in range(2)
                ]
                gh_ps = [
                    rzp.tile([128, 512], F32, name=f"gh{t}_{j}", tag=f"gh{j}")
                    for j in range(2)
                ]
                z_ps = [
                    rzp.tile([128, 512], F32, name=f"z{t}_{j}", tag=f"z{j}")
                    for j in range(2)
                ]

                def wih_open(ps, j4, j0):
                    # opens the j4 slice group (start zeroes the bank's
                    # write-bitmap: no other start may hit this bank until
                    # this group fully closes)
                    j = j0 + j4
                    oap = ps[j4 // 2][:, (j4 % 2) * 256 : (j4 % 2) * 256 + 256]
                    for k in range(HT):
                        nc.tensor.matmul(
                            oap, wih_sb[k][:, j * 128 : (j + 1) * 128],
                            xt[:, k * 256 : (k + 1) * 256],
                            start=(k == 0), stop=False)

                def whh_close(ps, j4, j0):
                    j = j0 + j4
                    oap = ps[j4 // 2][:, (j4 % 2) * 256 : (j4 % 2) * 256 + 256]
                    for k in range(HT):
                        nc.tensor.matmul(
                            oap, whh_sb[k][:, j * 128 : (j + 1) * 128],
                            s_cur[:, k * 256 : (k + 1) * 256],
                            start=False, stop=(k == HT - 1))

                def gh_bank(h):
                    for k4 in (2 * h, 2 * h + 1):
                        j = 8 + k4
                        oap = gh_ps[h][:, (k4 % 2) * 256 : (k4 % 2) * 256 + 256]
                        for k in range(HT):
                            nc.tensor.matmul(
                                oap, whh_sb[k][:, j * 128 : (j + 1) * 128],
                                s_cur[:, k * 256 : (k + 1) * 256],
                                start=(k == 0), stop=(k == HT - 1))

                # independent x-projections first (one open per bank),
                # then per-bank sequential close/open/close
                wih_open(r_ps, 0, 0)
                wih_open(r_ps, 2, 0)
                wih_open(z_ps, 0, 4)
                wih_open(z_ps, 2, 4)
                whh_close(r_ps, 0, 0)
                wih_open(r_ps, 1, 0)
                whh_close(r_ps, 1, 0)
                whh_close(r_ps, 2, 0)
                wih_open(r_ps, 3, 0)
                whh_close(r_ps, 3, 0)
                gh_bank(0)
                gh_bank(1)
                whh_close(z_ps, 0, 4)
                wih_open(z_ps, 1, 4)
                whh_close(z_ps, 1, 4)
                whh_close(z_ps, 2, 4)
                wih_open(z_ps, 3, 4)
                whh_close(z_ps, 3, 4)
                if t + 1 < T:
                    gx_nxt = prefill_gx(t + 1, xt_nxt)

                # ---- ACT: r sigmoids (bf16 out) ----
                r_t = sb.tile([128, SF], BF16, name=f"rt{t}", tag="rt", bufs=2)
                for k in range(HT):
                    nc.scalar.activation(
                        out=r_t[:, k * 256 : (k + 1) * 256],
                        in_=r_ps[k // 2][:, (k % 2) * 256 : (k % 2) * 256 + 256],
                        func=AF.Sigmoid, bias=brz_sb[:, k : k + 1], scale=1.0)

                # ---- DVE: w = (gh + bhn) * r  (per k-tile) ----
                w_t = sb.tile([128, SF], BF16, name=f"wt{t}", tag="wt", bufs=2)
                for k in range(HT):
                    nc.vector.scalar_tensor_tensor(
                        out=w_t[:, k * 256 : (k + 1) * 256],
                        in0=gh_ps[k // 2][:, (k % 2) * 256 : (k % 2) * 256 + 256],
                        scalar=bhn_sb[:, k : k + 1],
                        in1=r_t[:, k * 256 : (k + 1) * 256],
                        op0=ALU.add, op1=ALU.mult)

                # ---- DVE: u = w + gx (per X bank, [128,512]);
                #      Pool cannot read PSUM on TRN2 ----
                u_t = sb.tile([128, SF], BF16, name=f"ut{t}", tag="ut", bufs=2)
                for h in range(2):
                    nc.vector.tensor_add(
                        u_t[:, h * 512 : (h + 1) * 512],
                        w_t[:, h * 512 : (h + 1) * 512], gx_ps[h])

                # ---- ACT: tanh (per k-tile, bias=b_ih_n) / z sigmoids ----
                n_t = sb.tile([128, SF], BF16, name=f"nt{t}", tag="nt", bufs=2)
                z_t = sb.tile([128, SF], BF16, name=f"zt{t}", tag="zt", bufs=2)

                def tanh_k(k):
                    nc.scalar.activation(
                        out=n_t[:, k * 256 : (k + 1) * 256],
                        in_=u_t[:, k * 256 : (k + 1) * 256],
                        func=AF.Tanh, bias=bin_sb[:, k : k + 1], scale=1.0)

                def zsig_k(k):
                    nc.scalar.activation(
                        out=z_t[:, k * 256 : (k + 1) * 256],
                        in_=z_ps[k // 2][:, (k % 2) * 256 : (k % 2) * 256 + 256],
                        func=AF.Sigmoid, bias=brz_sb[:, 4 + k : 5 + k], scale=1.0)

                tanh_k(0)
                tanh_k(1)
                zsig_k(0)
                zsig_k(1)
                tanh_k(2)
                tanh_k(3)
                zsig_k(2)
                zsig_k(3)

                # ---- DVE chain (two [128,512] halves):
                #      t1 = s - n; t1 *= z; hn = t1 + n; s' = hn*mk;
                #      y = hn + x; y2 = y*y ----
                hn = sb.tile([128, SF], BF16, name=f"hn{t}", tag="hn", bufs=2)
                t1 = sb.tile([128, SF], BF16, name=f"t1{t}", tag="t1", bufs=2)
                s_nxt = None
                if t + 1 < T:
                    s_nxt = sb.tile([128, SF], BF16, name=f"s{t + 1}",
                                    tag="state", bufs=3)
                if main and toff == 0:
                    y_all = sb.tile([128, HT * FB], BF16,
                                    name=f"y{blk}", tag="y_all", bufs=2)
                    y2_all = sb.tile([128, HT * FB], BF16,
                                     name=f"y2{blk}", tag="y2_all", bufs=1)

                for h in range(2):
                    sl = slice(h * 512, (h + 1) * 512)
                    nc.vector.tensor_sub(t1[:, sl], s_cur[:, sl], n_t[:, sl])
                    nc.vector.tensor_mul(t1[:, sl], t1[:, sl], z_t[:, sl])
                    nc.vector.tensor_add(hn[:, sl], t1[:, sl], n_t[:, sl])
                    if s_nxt is not None:
                        for k in (2 * h, 2 * h + 1):
                            ksl = slice(k * 256, (k + 1) * 256)
                            nc.vector.tensor_mul(
                                s_nxt[:, ksl], hn[:, ksl], mk)
                    if main:
                        for k in (2 * h, 2 * h + 1):
                            o = k * FB + toff * S
                            ksl = slice(k * 256, (k + 1) * 256)
                            nc.vector.tensor_add(
                                y_all[:, o : o + S], hn[:, ksl], xt[:, ksl])
                            nc.gpsimd.tensor_mul(
                                y2_all[:, o : o + S],
                                y_all[:, o : o + S], y_all[:, o : o + S])

                if debug and t == 0:
                    pass
                    nc.sync.dma_start(out=dbg_d["r0"][:, :], in_=r_t)
                    nc.sync.dma_start(out=dbg_d["z0"][:, :], in_=z_t)
                    nc.sync.dma_start(out=dbg_d["n0"][:, :], in_=n_t)
                    nc.sync.dma_start(out=dbg_d["hn0"][:, :], in_=hn)
                    nc.sync.dma_start(out=dbg_d["s1"][:, :], in_=s_nxt)
                if debug and t == 1:
                    nc.sync.dma_start(out=dbg_d["s2x"][:, :], in_=s_nxt)
                if debug and t == T - 1:
                    nc.sync.dma_start(out=dbg_d["hnT"][:, :], in_=hn)

                # -- h0 injection at entry to main (chunk-0 columns) --
                if t + 1 == R:
                    inj = s_nxt.rearrange("p (k c) -> p k c", k=HT)[:, :, 0:S:C]
                    nc.vector.tensor_copy(
                        inj, h0m_sb.rearrange("p (k n) -> p k n", k=HT))

                # -- final hidden state (chunk C-1 columns) --
                if t == T - 1:
                    hl = hn.rearrange("p (k c) -> p k c", k=HT)[
                        :, :, C - 1 : S : C]
                    nc.sync.dma_start(out=hl_d[:, :], in_=hl)

                # ---- s2 (ACT tail): sqrt (costs 2 act-table loads) ----
                for dt_, b, st in stages:
                    if dt_ == 2:
                        sq = sb.tile([4, 512], F32, name=f"sq{b}", tag="sq",
                                     bufs=2)
                        nc.scalar.activation(
                            out=sq, in_=st["dvar"], func=AF.Sqrt,
                            bias=eps_sb[0:4, :], scale=1.0)
                        st["sq"] = sq

                # ---- yn stages (DVE tail): normalize one k-tile ----
                for dt_, b, st in stages:
                    if 5 <= dt_ <= 8:
                        k = dt_ - 5
                        yn = sb.tile([128, FB], BF16, name=f"yn{b}_{k}",
                                     tag="yn", bufs=2)
                        nc.vector.scalar_tensor_tensor(
                            out=yn, in0=st["y_all"][:, k * FB : (k + 1) * FB],
                            scalar=float(H), in1=st["mu_bc"],
                            op0=ALU.mult, op1=ALU.subtract)
                        nc.vector.tensor_mul(yn, yn, st["rs_bc"])
                        nc.gpsimd.tensor_scalar(
                            out=yn, in0=yn,
                            scalar1=gam_sb[:, k : k + 1],
                            scalar2=bet_sb[:, k : k + 1],
                            op0=ALU.mult, op1=ALU.add)
                        nc.sync.dma_start(
                            out=out_d[k, :, b * FB : (b + 1) * FB], in_=yn)
                        if k == HT - 1:
                            del ln[b]

                # ---- bc waves (PE tail): broadcast Smu/rs rows to
                #      [128,512] PSUM tiles via selector matmuls, borrowing
                #      the gh and z bank pairs (complete 1-mm groups) ----
                for dt_, b, st in stages:
                    if dt_ in (3, 4):
                        waves = (0, 1) if dt_ == 3 else (2, 3)
                        lst = []
                        for i, g in enumerate(waves):
                            tg = ("gh0", "gh1") if i == 0 else ("z0", "z1")
                            mu_ps = rzp.tile([128, 512], F32,
                                             name=f"bcm{b}_{g}", tag=tg[0])
                            rs_ps = rzp.tile([128, 512], F32,
                                             name=f"bcr{b}_{g}", tag=tg[1])
                            nc.tensor.matmul(
                                mu_ps, sel_sb[0:4, g * 128 : (g + 1) * 128],
                                st["pkm"], start=True, stop=True,
                                skip_group_check=True)
                            nc.tensor.matmul(
                                rs_ps, sel_sb[0:4, g * 128 : (g + 1) * 128],
                                st["pkr"], start=True, stop=True,
                                skip_group_check=True)
                            lst.append((g, mu_ps, rs_ps))
                        st[f"bcw{dt_ + 1}"] = lst

                # ---- blk_end (PE tail): LN column-sum matmuls ----
                if blk_end:
                    stp0 = rzp.tile([128, 512], F32, name=f"st{blk}a",
                                    tag="gh0")
                    stp1 = rzp.tile([128, 512], F32, name=f"st{blk}b",
                                    tag="gh1")
                    for g in range(4):   # Smu rows 0..3 <- y cols g*512..
                        for k in range(HT):
                            nc.tensor.matmul(
                                stp0[0:4, :],
                                ind_sb[:, g * 4 : (g + 1) * 4],
                                y_all[:, k * FB + g * 512 : k * FB + g * 512 + 512],
                                start=(g == 0 and k == 0),
                                stop=(g == 3 and k == HT - 1),
                                skip_group_check=True)
                    for g in range(4):   # Sss rows 0..3 <- y2
                        for k in range(HT):
                            nc.tensor.matmul(
                                stp1[0:4, :],
                                ind_sb[:, g * 4 : (g + 1) * 4],
                                y2_all[:, k * FB + g * 512 : k * FB + g * 512 + 512],
                                start=(g == 0 and k == 0),
                                stop=(g == 3 and k == HT - 1),
                                skip_group_check=True)
                    ln[blk] = {"te": t, "stp0": stp0, "stp1": stp1,
                               "y_all": y_all}

                if t + 1 < T:
                    s_cur = s_nxt
                    xt = xt_nxt
                    gx_ps = gx_nxt

            # ---- tail: finish LN for the last block(s) ----
            for b in sorted(ln):
                st = ln[b]
                st_mu = sb.tile([4, 512], F32, name=f"tstm{b}", tag="stm",
                                bufs=2)
                nc.scalar.activation(
                    out=st_mu, in_=st["stp0"][0:4, :], func=AF.Identity,
                    scale=1.0)
                st_ss = sb.tile([4, 512], F32, name=f"tstv{b}", tag="stv",
                                bufs=2)
                nc.scalar.activation(
                    out=st_ss, in_=st["stp1"][0:4, :], func=AF.Identity,
                    scale=1.0)
                musq = sb.tile([4, 512], F32, name=f"tmq{b}", tag="mq", bufs=2)
                nc.vector.tensor_mul(musq, st_mu, st_mu)
                dvar = sb.tile([4, 512], F32, name=f"tdv{b}", tag="dv", bufs=2)
                nc.vector.scalar_tensor_tensor(
                    out=dvar, in0=st_ss, scalar=float(H), in1=musq,
                    op0=ALU.mult, op1=ALU.subtract)
                sq = sb.tile([4, 512], F32, name=f"tsq{b}", tag="sq", bufs=2)
                nc.scalar.activation(
                    out=sq, in_=dvar, func=AF.Sqrt,
                    bias=eps_sb[0:4, :], scale=1.0)
                rst = sb.tile([4, 512], F32, name=f"trs{b}", tag="rs", bufs=2)
                nc.vector.reciprocal_approx_fast(out=rst, in_=sq)
                pkm = sb.tile([4, 512], BF16, name=f"tpkm{b}", tag="pkm",
                              bufs=2)
                nc.vector.tensor_copy(pkm, st_mu)
                pkr = sb.tile([4, 512], BF16, name=f"tpkr{b}", tag="pkr",
                              bufs=2)
                nc.vector.tensor_copy(pkr, rst)
                mu_bc = sb.tile([128, FB], BF16, name=f"tmubc{b}", tag="mubc",
                                bufs=1)
                rs_bc = sb.tile([128, FB], BF16, name=f"trsbc{b}", tag="rsbc",
                                bufs=1)
                tags = [("gh0", "gh1"), ("z0", "z1"), ("r0", "r1"),
                        ("gx0", "gx1")]
                for g in range(4):
                    tg = tags[g]
                    pool = gxp if tg[0].startswith("gx") else rzp
                    mu_ps = rzp.tile([128, 512], F32, name=f"tbm{b}_{g}",
                                     tag=tg[0]) if tg[0] != "gx0" else                         gxp.tile([128, 512], F32, name=f"tbm{b}_{g}",
                                 tag=tg[0])
                    rs_ps = rzp.tile([128, 512], F32, name=f"tbr{b}_{g}",
                                     tag=tg[1]) if tg[1] != "gx1" else                         gxp.tile([128, 512], F32, name=f"tbr{b}_{g}",
                                 tag=tg[1])
                    nc.tensor.matmul(
                        mu_ps, sel_sb[0:4, g * 128 : (g + 1) * 128], pkm,
                        start=True, stop=True, skip_group_check=True)
                    nc.tensor.matmul(
                        rs_ps, sel_sb[0:4, g * 128 : (g + 1) * 128], pkr,
                        start=True, stop=True, skip_group_check=True)
                    nc.scalar.activation(
                        out=mu_bc[:, g * 512 : (g + 1) * 512], in_=mu_ps,
                        func=AF.Identity, scale=1.0)
                    nc.scalar.activation(
                        out=rs_bc[:, g * 512 : (g + 1) * 512], in_=rs_ps,
                        func=AF.Identity, scale=1.0)
                for k in range(HT):
                    yn = sb.tile([128, FB], BF16, name=f"tyn{b}_{k}",
                                 tag="yn", bufs=2)
                    nc.vector.scalar_tensor_tensor(
                        out=yn, in0=st["y_all"][:, k * FB : (k + 1) * FB],
                        scalar=float(H), in1=mu_bc,
                        op0=ALU.mult, op1=ALU.subtract)
                    nc.vector.tensor_mul(yn, yn, rs_bc)
                    nc.gpsimd.tensor_scalar(
                        out=yn, in0=yn,
                        scalar1=gam_sb[:, k : k + 1],
                        scalar2=bet_sb[:, k : k + 1],
                        op0=ALU.mult, op1=ALU.add)
                    nc.sync.dma_start(
                        out=out_d[k, :, b * FB : (b + 1) * FB], in_=yn)
    nc.compile()
    return nc


def stage_inputs(input, h, is_initial, W_ih, W_hh, b_ih, b_hh, gamma, beta, R):
    """Host-side sharding/staging. Returns per-core input maps."""
    import ml_dtypes

    T = R + KS
    x = np.asarray(input, np.float32)
    h0 = np.asarray(h, np.float32)
    ii = np.asarray(is_initial).reshape(N, L)
    W_ih = np.asarray(W_ih, np.float32)
    W_hh = np.asarray(W_hh, np.float32)
    b_ih = np.asarray(b_ih, np.float32)
    b_hh = np.asarray(b_hh, np.float32)
    gamma = np.asarray(gamma, np.float32)
    beta = np.asarray(beta, np.float32)

    def bf(a):
        return np.ascontiguousarray(np.asarray(a, np.float32)).astype(
            ml_dtypes.bfloat16)

    mask = 1.0 - ii.astype(np.float32)  # [N, L]

    # l index per (c, t): warm-up reads the R steps before the chunk;
    # chunk 0's warm-up reads l in [KS-R, KS) (discarded garbage).
    l_for = np.empty((C, T), np.int64)
    for c in range(C):
        for t in range(T):
            l = c * KS + (t - R)
            l_for[c, t] = l if l >= 0 else l + KS

    wihT = np.ascontiguousarray(W_ih.T.reshape(HT, 128, 3 * H))
    whhT = np.ascontiguousarray(W_hh.T.reshape(HT, 128, 3 * H))
    brz = (b_ih + b_hh)[: 2 * H].reshape(8, 128).T.copy()        # [128, 8]
    bhn = b_hh[2 * H :].reshape(HT, 128).T.copy()                # [128, 4]
    binn = b_ih[2 * H :].reshape(HT, 128).T.copy()
    gam = gamma.reshape(HT, 128).T.copy()
    bet = beta.reshape(HT, 128).T.copy()
    # indicator stationary: group g (of 4) is a [128, 4] tile whose
    # column g is all-ones (routes a column-sum into PSUM partition g)
    ind = np.zeros((128, 16), np.float32)
    for g in range(4):
        ind[:, g * 4 + g] = 1.0
    # selector for the PSUM->all-partitions broadcast matmuls:
    # sel[c, g*128+po] = 1 iff c == g
    sel = np.zeros((4, 512), np.float32)
    for g in range(4):
        sel[g, g * 128 : (g + 1) * 128] = 1.0

    in_maps = []
    for core in range(NCORES):
        n0 = core * NB
        xc = x[n0 : n0 + NB]              # [NB, L, H]
        xg = xc[:, l_for, :]              # [NB, C, T, H]
        # xs2[t, p, k*S + s] with s = n*C + c, h = k*128 + p
        xs2 = np.ascontiguousarray(
            xg.transpose(2, 3, 0, 1)      # [T, H, NB, C]
            .reshape(T, HT, 128, S)
            .transpose(0, 2, 1, 3)        # [T, 128, HT, S]
            .reshape(T, 128, SF))
        mg = mask[n0 : n0 + NB][:, l_for]  # [NB, C, T]
        ms = np.ascontiguousarray(mg.transpose(2, 0, 1).reshape(T, S))
        m0 = mask[n0 : n0 + NB, 0]         # [NB]
        h0c = h0[n0 : n0 + NB] * m0[:, None]     # [NB, H]
        # h0m[p, k*NB + n] = h0c[n, k*128+p]
        h0m = np.ascontiguousarray(
            h0c.reshape(NB, HT, 128).transpose(2, 1, 0).reshape(128, HT * NB))
        in_maps.append({
            "xs": bf(xs2), "ms": bf(ms), "h0m": bf(h0m),
            "wih": bf(wihT), "whh": bf(whhT),
            "brz": brz, "bhn": bhn, "bin": binn,
            "gam": gam, "bet": bet, "ind": bf(ind), "sel": bf(sel),
        })
    return in_maps


def required_warmup(is_initial):
    """Max distance from a chunk boundary back to the latest reset."""
    ii = np.asarray(is_initial).reshape(N, L)
    need = 0
    for c in range(1, C):
        start = c * KS
        sub = ii[:, :start]
        for n in range(N):
            nz = np.nonzero(sub[n])[0]
            gap = start - nz[-1] if len(nz) else start
            need = max(need, gap)
    return need


def unstage_outputs(results):
    out = np.empty((N, L, H), np.float32)
    h_last = np.empty((N, H), np.float32)
    for core in range(NCORES):
        n0 = core * NB
        st = np.asarray(results[core]["out_st"], np.float32)  # [HT,128,KS*S]
        o = st.reshape(HT, 128, KS, NB, C).transpose(3, 4, 2, 0, 1)
        out[n0 : n0 + NB] = o.reshape(NB, L, H)
        hl = np.asarray(results[core]["hlast"], np.float32)   # [128, HT*NB]
        h_last[n0 : n0 + NB] = (
            hl.reshape(128, HT, NB).transpose(2, 1, 0).reshape(NB, H))
    h_exp = np.broadcast_to(h_last[:, None, :], (N, L, H)).copy()
    return out, h_exp


_PROGRAM_CACHE = {}


def kernel(input, h, is_initial, W_ih, W_hh, b_ih, b_hh, gamma, beta):
    need = required_warmup(is_initial)
    R = max(12, int(need))
    key = (R, True)
    if key not in _PROGRAM_CACHE:
        _PROGRAM_CACHE[key] = build_program(R)
    nc = _PROGRAM_CACHE[key]
    in_maps = stage_inputs(
        input, h, is_initial, W_ih, W_hh, b_ih, b_hh, gamma, beta, R)
    res = run_bass_kernel_spmd(nc, in_maps, list(range(NCORES))).results
    return unstage_outputs(res)


# revision 33
# speedup vs baseline: 1.4491x; 1.1061x over previous
"""Trainium2 Bass kernel for masked-GRU + residual + LayerNorm.

Problem: N=128 sequences of length L=512, hidden H=512.
  gx = x @ W_ih.T + b_ih            (precomputable input projection)
  per step l: hc = h * (1-is_initial[l]); gh = hc @ W_hh.T + b_hh
    r = sig(gx_r+gh_r); z = sig(gx_z+gh_z); n = tanh(gx_n + r*gh_n)
    h = (1-z)*n + z*hc
  out = LayerNorm(seq + x) * gamma + beta;  h_exp = broadcast(h_last)

Strategy (v2):
  * Data parallel: 16 batch rows per core (8 cores).
  * Sequence-chunk parallel: each L=512 sequence is split into C=16
    chunks of 32 steps, processed as independent columns, made exact by
    an R-step warm-up (a reset lands inside every R-window; verified at
    runtime). Chunk 0 injects the true h0 at entry to main.
  * bf16 everywhere off-PSUM: weights, x, gates, state, y, outputs.
    PSUM stays f32.  DVE runs 2-byte all-SBUF ops at 4x rate.
  * Packed state s [128, 1024] bf16 (4 h-tiles side by side in the free
    dim) so the elementwise chain runs as [128,512] halves.
  * PSUM banks (8): R pair (r gates), Z pair (z), G pair (gh_n),
    X pair (gx_n, lives across one step boundary).  LN stats borrow
    bank G0 right after it drains.
  * n-gate: gx_n stays in PSUM; w=(gh+bhh_n)*r on DVE, u=w+gx on Pool,
    tanh(u)+b_ih_n on ACT.  No identity-drain of gx.
  * LayerNorm per block of 8 steps, software-pipelined over the 7
    following steps so nothing blocks the recurrence: column sums via
    indicator-stationary matmuls into ONE [8,512] PSUM tile (rows 0-3
    Smu, rows 4-7 Sss), stats math as [4,512] multi-partition ops
    (D = H*Sss - Smu^2; rs = 1/sqrt(D + eps*H^2); out = ((H*y - Smu)
    * rs) * gamma + beta), DRAM-bounce broadcast of (Smu, rs) in bf16.
  * out / h_last stored bf16, upcast host-side.
"""
import sys

sys.path.insert(0, "/opt/trn_rl_repo")

import numpy as np

import concourse.bass as bass
import concourse.tile as tile
from concourse import bacc, mybir
from concourse.bass_utils import run_bass_kernel_spmd

F32 = mybir.dt.float32
BF16 = mybir.dt.bfloat16
AF = mybir.ActivationFunctionType
ALU = mybir.AluOpType

N, L, H = 128, 512, 512
NCORES = 8
NB = N // NCORES          # batch rows per core = 16
C = 16                    # chunks per sequence
KS = L // C               # main steps per chunk = 32
S = NB * C                # columns per core = 256
HT = H // 128             # h partition tiles = 4
BLK = 8                   # LN block (main steps)
NBLK = KS // BLK          # 4
FB = BLK * S              # 2048 block columns
SF = HT * S               # 1024 free dim of the packed state


def _bcast_ap(row_ap, parts=128, rep=1):
    """DRAM row AP -> partition-broadcast AP (0-stride over partitions,
    optionally replicated rep times along free)."""
    ap = [[0, parts]]
    if rep > 1:
        ap.append([0, rep])
    ap += [list(d) for d in row_ap.ap]
    return bass.AP(tensor=row_ap.tensor, offset=row_ap.offset, ap=ap)


def build_program(R=12, debug=False):
    T = R + KS
    nc = bacc.Bacc("TRN2", target_bir_lowering=False)

    xs_d = nc.declare_dram_parameter("xs", [T, 128, SF], BF16, isOutput=False)
    ms_d = nc.declare_dram_parameter("ms", [T, S], BF16, isOutput=False)
    h0m_d = nc.declare_dram_parameter("h0m", [128, HT * NB], BF16, isOutput=False)
    wih_d = nc.declare_dram_parameter("wih", [HT, 128, 3 * H], BF16, isOutput=False)
    whh_d = nc.declare_dram_parameter("whh", [HT, 128, 3 * H], BF16, isOutput=False)
    brz_d = nc.declare_dram_parameter("brz", [128, 8], F32, isOutput=False)
    bhn_d = nc.declare_dram_parameter("bhn", [128, HT], F32, isOutput=False)
    bin_d = nc.declare_dram_parameter("bin", [128, HT], F32, isOutput=False)
    gam_d = nc.declare_dram_parameter("gam", [128, HT], F32, isOutput=False)
    bet_d = nc.declare_dram_parameter("bet", [128, HT], F32, isOutput=False)
    ind_d = nc.declare_dram_parameter("ind", [128, 16], BF16, isOutput=False)
    sel_d = nc.declare_dram_parameter("sel", [4, 512], BF16, isOutput=False)

    out_d = nc.declare_dram_parameter("out_st", [HT, 128, KS * S], BF16, isOutput=True)
    if debug:
        dbg_names = ["mk0", "r0", "z0", "n0", "hn0", "s1", "s2x", "hnT"]
        dbg_d = {
            nm: nc.declare_dram_parameter(f"dbg_{nm}", [128, SF], BF16,
                                          isOutput=True)
            for nm in dbg_names
        }
    hl_d = nc.declare_dram_parameter("hlast", [128, HT * NB], BF16, isOutput=True)


    with tile.TileContext(nc) as tc:
        with (
            tc.tile_pool(name="const", bufs=1) as cst,
            tc.tile_pool(name="sb", bufs=1) as sb,
            tc.tile_pool(name="rzp", bufs=1, space="PSUM") as rzp,
            tc.tile_pool(name="gxp", bufs=1, space="PSUM") as gxp,
        ):
            # ---- constants (wih first: needed by the t=0 prefill) ----
            wih_sb, whh_sb = [], []
            for k in range(HT):
                w1 = cst.tile([128, 3 * H], BF16, name=f"wih_sb{k}", tag=f"wih{k}")
                nc.sync.dma_start(out=w1, in_=wih_d[k, :, :])
                wih_sb.append(w1)
            x0 = sb.tile([128, SF], BF16, name="xt0", tag="xt", bufs=3)
            nc.sync.dma_start(out=x0, in_=xs_d[0, :, :])
            for k in range(HT):
                w2 = cst.tile([128, 3 * H], BF16, name=f"whh_sb{k}", tag=f"whh{k}")
                nc.sync.dma_start(out=w2, in_=whh_d[k, :, :])
                whh_sb.append(w2)
            h0m_sb = cst.tile([128, HT * NB], BF16, name="h0m_sb", tag="h0m")
            nc.sync.dma_start(out=h0m_sb, in_=h0m_d[:, :])
            brz_sb = cst.tile([128, 8], F32, name="brz_sb", tag="brz")
            nc.sync.dma_start(out=brz_sb, in_=brz_d[:, :])
            bhn_sb = cst.tile([128, HT], F32, name="bhn_sb", tag="bhn")
            nc.sync.dma_start(out=bhn_sb, in_=bhn_d[:, :])
            bin_sb = cst.tile([128, HT], F32, name="bin_sb", tag="bin")
            nc.sync.dma_start(out=bin_sb, in_=bin_d[:, :])
            gam_sb = cst.tile([128, HT], F32, name="gam_sb", tag="gam")
            nc.sync.dma_start(out=gam_sb, in_=gam_d[:, :])
            bet_sb = cst.tile([128, HT], F32, name="bet_sb", tag="bet")
            nc.sync.dma_start(out=bet_sb, in_=bet_d[:, :])
            ind_sb = cst.tile([128, 16], BF16, name="ind_sb", tag="ind")
            nc.sync.dma_start(out=ind_sb, in_=ind_d[:, :])
            eps_sb = cst.tile([128, 1], F32, name="eps_sb", tag="eps")
            nc.vector.memset(eps_sb, float(H) * float(H) * 1e-5)
            sel_sb = cst.tile([4, 512], BF16, name="sel_sb", tag="sel")
            nc.sync.dma_start(out=sel_sb, in_=sel_d[:, :])
            # masks: load once to partition 0, broadcast on Pool
            ms_all = cst.tile([128, T * S], BF16, name="ms_all", tag="msb")
            nc.sync.dma_start(
                out=ms_all[0:1, :], in_=ms_d[:, :].rearrange("t s -> (t s)"))
            nc.gpsimd.partition_broadcast(ms_all, ms_all[0:1, :])

            # ---- initial (zero) state ----
            s_cur = sb.tile([128, SF], BF16, name="s_init", tag="state", bufs=3)
            nc.vector.memset(s_cur, 0.0)

            def load_x(t):
                xt = sb.tile([128, SF], BF16, name=f"xt{t}", tag="xt", bufs=3)
                nc.sync.dma_start(out=xt, in_=xs_d[t, :, :])
                return xt



            def prefill_gx(t, xt):
                # open+close gx_n groups for step t (wih only); X pair
                # [gx8|gx9], [gx10|gx11]; stays in PSUM until step t's u.
                gx_ps = [
                    gxp.tile([128, 512], F32, name=f"gx{t}_{j}", tag=f"gx{j}")
                    for j in range(2)
                ]
                for k4 in range(4):
                    j = 8 + k4
                    oap = gx_ps[k4 // 2][:, (k4 % 2) * 256 : (k4 % 2) * 256 + 256]
                    for k in range(HT):
                        nc.tensor.matmul(
                            oap, wih_sb[k][:, j * 128 : (j + 1) * 128],
                            xt[:, k * 256 : (k + 1) * 256],
                            start=(k == 0), stop=(k == HT - 1))
                return gx_ps

            xt = x0
            gx_ps = prefill_gx(0, xt)

            # LN pipeline state, keyed by block id
            ln = {}
            y_all = y2_all = None

            for t in range(T):
                main = t >= R
                toff = (t - R) % BLK
                blk = (t - R) // BLK
                blk_end = main and toff == BLK - 1

                # LN pipeline stage for earlier blocks this iteration
                stages = [(t - st["te"], b, st) for b, st in list(ln.items())
                          if 1 <= t - st["te"] <= 8]

                # ---- s1 (ACT top): drain the stats PSUM banks ----
                for dt_, b, st in stages:
                    if dt_ == 1:
                        st_mu = sb.tile([4, 512], F32, name=f"stm{b}",
                                        tag="stm", bufs=2)
                        nc.scalar.activation(
                            out=st_mu, in_=st["stp0"][0:4, :],
                            func=AF.Identity, scale=1.0)
                        st_ss = sb.tile([4, 512], F32, name=f"stv{b}",
                                        tag="stv", bufs=2)
                        nc.scalar.activation(
                            out=st_ss, in_=st["stp1"][0:4, :],
                            func=AF.Identity, scale=1.0)
                        st["st_mu"] = st_mu
                        st["st_ss"] = st_ss

                if t + 1 < T:
                    xt_nxt = load_x(t + 1)
                    mk = ms_all[:, (t + 1) * S : (t + 2) * S]

                # ---- s1 (DVE top): musq, dvar ----
                for dt_, b, st in stages:
                    if dt_ == 1:
                        musq = sb.tile([4, 512], F32, name=f"mq{b}", tag="mq",
                                       bufs=2)
                        nc.vector.tensor_mul(
                            musq, st["st_mu"], st["st_mu"])
                        dvar = sb.tile([4, 512], F32, name=f"dv{b}", tag="dv",
                                       bufs=2)
                        nc.vector.scalar_tensor_tensor(
                            out=dvar, in0=st["st_ss"],
                            scalar=float(H), in1=musq,
                            op0=ALU.mult, op1=ALU.subtract)
                        st["dvar"] = dvar

                # ---- s3 (DVE top): recip + bf16 packs ----
                for dt_, b, st in stages:
                    if dt_ == 3:
                        rst = sb.tile([4, 512], F32, name=f"rs{b}", tag="rs",
                                      bufs=2)
                        nc.vector.reciprocal_approx_fast(out=rst, in_=st["sq"])
                        pkm = sb.tile([4, 512], BF16, name=f"pkm{b}",
                                      tag="pkm", bufs=2)
                        nc.vector.tensor_copy(pkm, st["st_mu"])
                        pkr = sb.tile([4, 512], BF16, name=f"pkr{b}",
                                      tag="pkr", bufs=2)
                        nc.vector.tensor_copy(pkr, rst)
                        st["pkm"] = pkm
                        st["pkr"] = pkr
                        st["mu_bc"] = sb.tile([128, FB], BF16, name=f"mubc{b}",
                                              tag="mubc", bufs=1)
                        st["rs_bc"] = sb.tile([128, FB], BF16, name=f"rsbc{b}",
                                              tag="rsbc", bufs=1)



                # ---- PE: all r/z/gh groups open and close within this
                #      iteration (cross-iteration open groups on sliced
                #      tiles miscompile).  Order: independent wih opens
                #      first, state-dependent whh closes mid-stream, the
                #      complete gx prefill for t+1 last. ----
                r_ps = [
                    rzp.tile([128, 512], F32, name=f"r{t}_{j}", tag=f"r{j}")
                    for j in range(2)
                ]
                gh_ps = [
                    rzp.tile([128, 512], F32, name=f"gh{t}_{j}", tag=f"gh{j}")
                    for j in range(2)
                ]
                z_ps = [
                    rzp.tile([128, 512], F32, name=f"z{t}_{j}", tag=f"z{j}")
                    for j in range(2)
                ]

                def wih_open(ps, j4, j0):
                    # opens the j4 slice group (start zeroes the bank's
                    # write-bitmap: no other start may hit this bank until
                    # this group fully closes)
                    j = j0 + j4
                    oap = ps[j4 // 2][:, (j4 % 2) * 256 : (j4 % 2) * 256 + 256]
                    for k in range(HT):
                        nc.tensor.matmul(
                            oap, wih_sb[k][:, j * 128 : (j + 1) * 128],
                            xt[:, k * 256 : (k + 1) * 256],
                            start=(k == 0), stop=False)

                def whh_close(ps, j4, j0):
                    j = j0 + j4
                    oap = ps[j4 // 2][:, (j4 % 2) * 256 : (j4 % 2) * 256 + 256]
                    for k in range(HT):
                        nc.tensor.matmul(
                            oap, whh_sb[k][:, j * 128 : (j + 1) * 128],
                            s_cur[:, k * 256 : (k + 1) * 256],
                            start=False, stop=(k == HT - 1))

                def gh_bank(h):
                    for k4 in (2 * h, 2 * h + 1):
                        j = 8 + k4
                        oap = gh_ps[h][:, (k4 % 2) * 256 : (k4 % 2) * 256 + 256]
                        for k in range(HT):
                            nc.tensor.matmul(
                                oap, whh_sb[k][:, j * 128 : (j + 1) * 128],
                                s_cur[:, k * 256 : (k + 1) * 256],
                                start=(k == 0), stop=(k == HT - 1))

                # independent x-projections first (one open per bank),
                # then per-bank sequential close/open/close
                wih_open(r_ps, 0, 0)
                wih_open(r_ps, 2, 0)
                wih_open(z_ps, 0, 4)
                wih_open(z_ps, 2, 4)
                whh_close(r_ps, 0, 0)
                wih_open(r_ps, 1, 0)
                whh_close(r_ps, 1, 0)
                whh_close(r_ps, 2, 0)
                wih_open(r_ps, 3, 0)
                whh_close(r_ps, 3, 0)
                gh_bank(0)
                gh_bank(1)
                whh_close(z_ps, 0, 4)
                wih_open(z_ps, 1, 4)
                whh_close(z_ps, 1, 4)
                whh_close(z_ps, 2, 4)
                wih_open(z_ps, 3, 4)
                whh_close(z_ps, 3, 4)
                if t + 1 < T:
                    gx_nxt = prefill_gx(t + 1, xt_nxt)

                # ---- ACT: r sigmoids (bf16 out) ----
                r_t = sb.tile([128, SF], BF16, name=f"rt{t}", tag="rt", bufs=2)
                for k in range(HT):
                    nc.scalar.activation(
                        out=r_t[:, k * 256 : (k + 1) * 256],
                        in_=r_ps[k // 2][:, (k % 2) * 256 : (k % 2) * 256 + 256],
                        func=AF.Sigmoid, bias=brz_sb[:, k : k + 1], scale=1.0)

                # ---- DVE: w = (gh + bhn) * r  (per k-tile) ----
                w_t = sb.tile([128, SF], BF16, name=f"wt{t}", tag="wt", bufs=2)
                for k in range(HT):
                    nc.vector.scalar_tensor_tensor(
                        out=w_t[:, k * 256 : (k + 1) * 256],
                        in0=gh_ps[k // 2][:, (k % 2) * 256 : (k % 2) * 256 + 256],
                        scalar=bhn_sb[:, k : k + 1],
                        in1=r_t[:, k * 256 : (k + 1) * 256],
                        op0=ALU.add, op1=ALU.mult)

                # ---- DVE: u = w + gx (per X bank, [128,512]);
                #      Pool cannot read PSUM on TRN2 ----
                u_t = sb.tile([128, SF], BF16, name=f"ut{t}", tag="ut", bufs=2)
                for h in range(2):
                    nc.vector.tensor_add(
                        u_t[:, h * 512 : (h + 1) * 512],
                        w_t[:, h * 512 : (h + 1) * 512], gx_ps[h])

                # ---- ACT: tanh (per k-tile, bias=b_ih_n) / z sigmoids ----
                n_t = sb.tile([128, SF], BF16, name=f"nt{t}", tag="nt", bufs=2)
                z_t = sb.tile([128, SF], BF16, name=f"zt{t}", tag="zt", bufs=2)

                def tanh_k(k):
                    nc.scalar.activation(
                        out=n_t[:, k * 256 : (k + 1) * 256],
                        in_=u_t[:, k * 256 : (k + 1) * 256],
                        func=AF.Tanh, bias=bin_sb[:, k : k + 1], scale=1.0)

                def zsig_k(k):
                    nc.scalar.activation(
                        out=z_t[:, k * 256 : (k + 1) * 256],
                        in_=z_ps[k // 2][:, (k % 2) * 256 : (k % 2) * 256 + 256],
                        func=AF.Sigmoid, bias=brz_sb[:, 4 + k : 5 + k], scale=1.0)

                tanh_k(0)
                tanh_k(1)
                zsig_k(0)
                zsig_k(1)
                tanh_k(2)
                tanh_k(3)
                zsig_k(2)
                zsig_k(3)

                # ---- DVE chain (two [128,512] halves):
                #      t1 = s - n; t1 *= z; hn = t1 + n; s' = hn*mk;
                #      y = hn + x; y2 = y*y ----
                hn = sb.tile([128, SF], BF16, name=f"hn{t}", tag="hn", bufs=2)
                t1 = sb.tile([128, SF], BF16, name=f"t1{t}", tag="t1", bufs=2)
                s_nxt = None
                if t + 1 < T:
                    s_nxt = sb.tile([128, SF], BF16, name=f"s{t + 1}",
                                    tag="state", bufs=3)
                if main and toff == 0:
                    y_all = sb.tile([128, HT * FB], BF16,
                                    name=f"y{blk}", tag="y_all", bufs=2)
                    y2_all = sb.tile([128, HT * FB], BF16,
                                     name=f"y2{blk}", tag="y2_all", bufs=1)

                for h in range(2):
                    sl = slice(h * 512, (h + 1) * 512)
                    nc.vector.tensor_sub(t1[:, sl], s_cur[:, sl], n_t[:, sl])
                    nc.vector.tensor_mul(t1[:, sl], t1[:, sl], z_t[:, sl])
                    nc.vector.tensor_add(hn[:, sl], t1[:, sl], n_t[:, sl])
                    if s_nxt is not None:
                        for k in (2 * h, 2 * h + 1):
                            ksl = slice(k * 256, (k + 1) * 256)
                            nc.vector.tensor_mul(
                                s_nxt[:, ksl], hn[:, ksl], mk)
                    if main:
                        for k in (2 * h, 2 * h + 1):
                            o = k * FB + toff * S
                            ksl = slice(k * 256, (k + 1) * 256)
                            nc.vector.tensor_add(
                                y_all[:, o : o + S], hn[:, ksl], xt[:, ksl])
                            nc.gpsimd.tensor_mul(
                                y2_all[:, o : o + S],
                                y_all[:, o : o + S], y_all[:, o : o + S])

                if debug and t == 0:
                    pass
                    nc.sync.dma_start(out=dbg_d["r0"][:, :], in_=r_t)
                    nc.sync.dma_start(out=dbg_d["z0"][:, :], in_=z_t)
                    nc.sync.dma_start(out=dbg_d["n0"][:, :], in_=n_t)
                    nc.sync.dma_start(out=dbg_d["hn0"][:, :], in_=hn)
                    nc.sync.dma_start(out=dbg_d["s1"][:, :], in_=s_nxt)
                if debug and t == 1:
                    nc.sync.dma_start(out=dbg_d["s2x"][:, :], in_=s_nxt)
                if debug and t == T - 1:
                    nc.sync.dma_start(out=dbg_d["hnT"][:, :], in_=hn)

                # -- h0 injection at entry to main (chunk-0 columns) --
                if t + 1 == R:
                    inj = s_nxt.rearrange("p (k c) -> p k c", k=HT)[:, :, 0:S:C]
                    nc.vector.tensor_copy(
                        inj, h0m_sb.rearrange("p (k n) -> p k n", k=HT))

                # -- final hidden state (chunk C-1 columns); compact on
                #    DVE first (a strided DMA lowers to 2-byte packets) --
                if t == T - 1:
                    hl = hn.rearrange("p (k c) -> p k c", k=HT)[
                        :, :, C - 1 : S : C]
                    hlc = sb.tile([128, HT * NB], BF16, name="hlc", tag="hlc")
                    nc.vector.tensor_copy(
                        hlc.rearrange("p (k n) -> p k n", k=HT), hl)
                    nc.sync.dma_start(out=hl_d[:, :], in_=hlc)

                # ---- s2 (ACT tail): sqrt (costs 2 act-table loads) ----
                for dt_, b, st in stages:
                    if dt_ == 2:
                        sq = sb.tile([4, 512], F32, name=f"sq{b}", tag="sq",
                                     bufs=2)
                        nc.scalar.activation(
                            out=sq, in_=st["dvar"], func=AF.Sqrt,
                            bias=eps_sb[0:4, :], scale=1.0)
                        st["sq"] = sq

                # ---- yn stages (DVE tail): normalize one k-tile ----
                for dt_, b, st in stages:
                    if 5 <= dt_ <= 8:
                        k = dt_ - 5
                        yn = sb.tile([128, FB], BF16, name=f"yn{b}_{k}",
                                     tag="yn", bufs=2)
                        nc.vector.scalar_tensor_tensor(
                            out=yn, in0=st["y_all"][:, k * FB : (k + 1) * FB],
                            scalar=float(H), in1=st["mu_bc"],
                            op0=ALU.mult, op1=ALU.subtract)
                        nc.vector.tensor_mul(yn, yn, st["rs_bc"])
                        nc.gpsimd.tensor_scalar(
                            out=yn, in0=yn,
                            scalar1=gam_sb[:, k : k + 1],
                            scalar2=bet_sb[:, k : k + 1],
                            op0=ALU.mult, op1=ALU.add)
                        nc.sync.dma_start(
                            out=out_d[k, :, b * FB : (b + 1) * FB], in_=yn)
                        if k == HT - 1:
                            del ln[b]

                # ---- bc waves (PE tail): broadcast Smu/rs rows to
                #      [128,512] PSUM tiles via selector matmuls, borrowing
                #      the gh and z bank pairs (complete 1-mm groups) ----
                for dt_, b, st in stages:
                    if dt_ in (3, 4):
                        waves = (0, 1) if dt_ == 3 else (2, 3)
                        for i, g in enumerate(waves):
                            tg = ("gh0", "gh1") if i == 0 else ("r0", "r1")
                            mu_ps = rzp.tile([128, 512], F32,
                                             name=f"bcm{b}_{g}", tag=tg[0])
                            rs_ps = rzp.tile([128, 512], F32,
                                             name=f"bcr{b}_{g}", tag=tg[1])
                            nc.tensor.matmul(
                                mu_ps, sel_sb[0:4, g * 128 : (g + 1) * 128],
                                st["pkm"], start=True, stop=True,
                                skip_group_check=True)
                            nc.tensor.matmul(
                                rs_ps, sel_sb[0:4, g * 128 : (g + 1) * 128],
                                st["pkr"], start=True, stop=True,
                                skip_group_check=True)
                            nc.scalar.activation(
                                out=st["mu_bc"][:, g * 512 : (g + 1) * 512],
                                in_=mu_ps, func=AF.Identity, scale=1.0)
                            nc.scalar.activation(
                                out=st["rs_bc"][:, g * 512 : (g + 1) * 512],
                                in_=rs_ps, func=AF.Identity, scale=1.0)

                # ---- blk_end (PE tail): LN column-sum matmuls ----
                if blk_end:
                    stp0 = rzp.tile([128, 512], F32, name=f"st{blk}a",
                                    tag="gh0")
                    stp1 = rzp.tile([128, 512], F32, name=f"st{blk}b",
                                    tag="gh1")
                    for g in range(4):   # Smu rows 0..3 <- y cols g*512..
                        for k in range(HT):
                            nc.tensor.matmul(
                                stp0[0:4, :],
                                ind_sb[:, g * 4 : (g + 1) * 4],
                                y_all[:, k * FB + g * 512 : k * FB + g * 512 + 512],
                                start=(g == 0 and k == 0),
                                stop=(g == 3 and k == HT - 1),
                                skip_group_check=True)
                    for g in range(4):   # Sss rows 0..3 <- y2
                        for k in range(HT):
                            nc.tensor.matmul(
                                stp1[0:4, :],
                                ind_sb[:, g * 4 : (g + 1) * 4],
                                y2_all[:, k * FB + g * 512 : k * FB + g * 512 + 512],
                                start=(g == 0 and k == 0),
                                stop=(g == 3 and k == HT - 1),
                                skip_group_check=True)
                    ln[blk] = {"te": t, "stp0": stp0, "stp1": stp1,
                               "y_all": y_all}

                if t + 1 < T:
                    s_cur = s_nxt
                    xt = xt_nxt
                    gx_ps = gx_nxt

            # ---- tail: finish LN for the last block(s) ----
            for b in sorted(ln):
                st = ln[b]
                st_mu = sb.tile([4, 512], F32, name=f"tstm{b}", tag="stm",
                                bufs=2)
                nc.scalar.activation(
                    out=st_mu, in_=st["stp0"][0:4, :], func=AF.Identity,
                    scale=1.0)
                st_ss = sb.tile([4, 512], F32, name=f"tstv{b}", tag="stv",
                                bufs=2)
                nc.scalar.activation(
                    out=st_ss, in_=st["stp1"][0:4, :], func=AF.Identity,
                    scale=1.0)
                musq = sb.tile([4, 512], F32, name=f"tmq{b}", tag="mq", bufs=2)
                nc.vector.tensor_mul(musq, st_mu, st_mu)
                dvar = sb.tile([4, 512], F32, name=f"tdv{b}", tag="dv", bufs=2)
                nc.vector.scalar_tensor_tensor(
                    out=dvar, in0=st_ss, scalar=float(H), in1=musq,
                    op0=ALU.mult, op1=ALU.subtract)
                sq = sb.tile([4, 512], F32, name=f"tsq{b}", tag="sq", bufs=2)
                nc.scalar.activation(
                    out=sq, in_=dvar, func=AF.Sqrt,
                    bias=eps_sb[0:4, :], scale=1.0)
                rst = sb.tile([4, 512], F32, name=f"trs{b}", tag="rs", bufs=2)
                nc.vector.reciprocal_approx_fast(out=rst, in_=sq)
                pkm = sb.tile([4, 512], BF16, name=f"tpkm{b}", tag="pkm",
                              bufs=2)
                nc.vector.tensor_copy(pkm, st_mu)
                pkr = sb.tile([4, 512], BF16, name=f"tpkr{b}", tag="pkr",
                              bufs=2)
                nc.vector.tensor_copy(pkr, rst)
                mu_bc = sb.tile([128, FB], BF16, name=f"tmubc{b}", tag="mubc",
                                bufs=1)
                rs_bc = sb.tile([128, FB], BF16, name=f"trsbc{b}", tag="rsbc",
                                bufs=1)
                tags = [("gh0", "gh1"), ("z0", "z1"), ("r0", "r1"),
                        ("gx0", "gx1")]
                for g in range(4):
                    tg = tags[g]
                    pool = gxp if tg[0].startswith("gx") else rzp
                    mu_ps = rzp.tile([128, 512], F32, name=f"tbm{b}_{g}",
                                     tag=tg[0]) if tg[0] != "gx0" else                         gxp.tile([128, 512], F32, name=f"tbm{b}_{g}",
                                 tag=tg[0])
                    rs_ps = rzp.tile([128, 512], F32, name=f"tbr{b}_{g}",
                                     tag=tg[1]) if tg[1] != "gx1" else                         gxp.tile([128, 512], F32, name=f"tbr{b}_{g}",
                                 tag=tg[1])
                    nc.tensor.matmul(
                        mu_ps, sel_sb[0:4, g * 128 : (g + 1) * 128], pkm,
                        start=True, stop=True, skip_group_check=True)
                    nc.tensor.matmul(
                        rs_ps, sel_sb[0:4, g * 128 : (g + 1) * 128], pkr,
                        start=True, stop=True, skip_group_check=True)
                    nc.scalar.activation(
                        out=mu_bc[:, g * 512 : (g + 1) * 512], in_=mu_ps,
                        func=AF.Identity, scale=1.0)
                    nc.scalar.activation(
                        out=rs_bc[:, g * 512 : (g + 1) * 512], in_=rs_ps,
                        func=AF.Identity, scale=1.0)
                for k in range(HT):
                    yn = sb.tile([128, FB], BF16, name=f"tyn{b}_{k}",
                                 tag="yn", bufs=2)
                    nc.vector.scalar_tensor_tensor(
                        out=yn, in0=st["y_all"][:, k * FB : (k + 1) * FB],
                        scalar=float(H), in1=mu_bc,
                        op0=ALU.mult, op1=ALU.subtract)
                    nc.vector.tensor_mul(yn, yn, rs_bc)
                    nc.gpsimd.tensor_scalar(
                        out=yn, in0=yn,
                        scalar1=gam_sb[:, k : k + 1],
                        scalar2=bet_sb[:, k : k + 1],
                        op0=ALU.mult, op1=ALU.add)
                    nc.sync.dma_start(
                        out=out_d[k, :, b * FB : (b + 1) * FB], in_=yn)
    nc.compile()
    return nc


def stage_inputs(input, h, is_initial, W_ih, W_hh, b_ih, b_hh, gamma, beta, R):
    """Host-side sharding/staging. Returns per-core input maps."""
    import ml_dtypes

    T = R + KS
    x = np.asarray(input, np.float32)
    h0 = np.asarray(h, np.float32)
    ii = np.asarray(is_initial).reshape(N, L)
    W_ih = np.asarray(W_ih, np.float32)
    W_hh = np.asarray(W_hh, np.float32)
    b_ih = np.asarray(b_ih, np.float32)
    b_hh = np.asarray(b_hh, np.float32)
    gamma = np.asarray(gamma, np.float32)
    beta = np.asarray(beta, np.float32)

    def bf(a):
        return np.ascontiguousarray(np.asarray(a, np.float32)).astype(
            ml_dtypes.bfloat16)

    mask = 1.0 - ii.astype(np.float32)  # [N, L]

    # l index per (c, t): warm-up reads the R steps before the chunk;
    # chunk 0's warm-up reads l in [KS-R, KS) (discarded garbage).
    l_for = np.empty((C, T), np.int64)
    for c in range(C):
        for t in range(T):
            l = c * KS + (t - R)
            l_for[c, t] = l if l >= 0 else l + KS

    wihT = np.ascontiguousarray(W_ih.T.reshape(HT, 128, 3 * H))
    whhT = np.ascontiguousarray(W_hh.T.reshape(HT, 128, 3 * H))
    brz = (b_ih + b_hh)[: 2 * H].reshape(8, 128).T.copy()        # [128, 8]
    bhn = b_hh[2 * H :].reshape(HT, 128).T.copy()                # [128, 4]
    binn = b_ih[2 * H :].reshape(HT, 128).T.copy()
    gam = gamma.reshape(HT, 128).T.copy()
    bet = beta.reshape(HT, 128).T.copy()
    # indicator stationary: group g (of 4) is a [128, 4] tile whose
    # column g is all-ones (routes a column-sum into PSUM partition g)
    ind = np.zeros((128, 16), np.float32)
    for g in range(4):
        ind[:, g * 4 + g] = 1.0
    # selector for the PSUM->all-partitions broadcast matmuls:
    # sel[c, g*128+po] = 1 iff c == g
    sel = np.zeros((4, 512), np.float32)
    for g in range(4):
        sel[g, g * 128 : (g + 1) * 128] = 1.0

    in_maps = []
    for core in range(NCORES):
        n0 = core * NB
        xc = x[n0 : n0 + NB]              # [NB, L, H]
        xg = xc[:, l_for, :]              # [NB, C, T, H]
        # xs2[t, p, k*S + s] with s = n*C + c, h = k*128 + p
        xs2 = np.ascontiguousarray(
            xg.transpose(2, 3, 0, 1)      # [T, H, NB, C]
            .reshape(T, HT, 128, S)
            .transpose(0, 2, 1, 3)        # [T, 128, HT, S]
            .reshape(T, 128, SF))
        mg = mask[n0 : n0 + NB][:, l_for]  # [NB, C, T]
        ms = np.ascontiguousarray(mg.transpose(2, 0, 1).reshape(T, S))
        m0 = mask[n0 : n0 + NB, 0]         # [NB]
        h0c = h0[n0 : n0 + NB] * m0[:, None]     # [NB, H]
        # h0m[p, k*NB + n] = h0c[n, k*128+p]
        h0m = np.ascontiguousarray(
            h0c.reshape(NB, HT, 128).transpose(2, 1, 0).reshape(128, HT * NB))
        in_maps.append({
            "xs": bf(xs2), "ms": bf(ms), "h0m": bf(h0m),
            "wih": bf(wihT), "whh": bf(whhT),
            "brz": brz, "bhn": bhn, "bin": binn,
            "gam": gam, "bet": bet, "ind": bf(ind), "sel": bf(sel),
        })
    return in_maps


def required_warmup(is_initial):
    """Max distance from a chunk boundary back to the latest reset."""
    ii = np.asarray(is_initial).reshape(N, L)
    need = 0
    for c in range(1, C):
        start = c * KS
        sub = ii[:, :start]
        for n in range(N):
            nz = np.nonzero(sub[n])[0]
            gap = start - nz[-1] if len(nz) else start
            need = max(need, gap)
    return need


def unstage_outputs(results):
    out = np.empty((N, L, H), np.float32)
    h_last = np.empty((N, H), np.float32)
    for core in range(NCORES):
        n0 = core * NB
        st = np.asarray(results[core]["out_st"], np.float32)  # [HT,128,KS*S]
        o = st.reshape(HT, 128, KS, NB, C).transpose(3, 4, 2, 0, 1)
        out[n0 : n0 + NB] = o.reshape(NB, L, H)
        hl = np.asarray(results[core]["hlast"], np.float32)   # [128, HT*NB]
        h_last[n0 : n0 + NB] = (
            hl.reshape(128, HT, NB).transpose(2, 1, 0).reshape(NB, H))
    h_exp = np.broadcast_to(h_last[:, None, :], (N, L, H)).copy()
    return out, h_exp


_PROGRAM_CACHE = {}


def kernel(input, h, is_initial, W_ih, W_hh, b_ih, b_hh, gamma, beta):
    need = required_warmup(is_initial)
    R = max(12, int(need))
    key = (R, True)
    if key not in _PROGRAM_CACHE:
        _PROGRAM_CACHE[key] = build_program(R)
    nc = _PROGRAM_CACHE[key]
    in_maps = stage_inputs(
        input, h, is_initial, W_ih, W_hh, b_ih, b_hh, gamma, beta, R)
    res = run_bass_kernel_spmd(nc, in_maps, list(range(NCORES))).results
    return unstage_outputs(res)
